# revision 1
# baseline (speedup 1.0000x reference)
"""Trainium2 Bass kernel for a causal-attention-like module.

Math (reassociated from the reference nn.Module):
    dist[i,j] = sqrt(max(|T_i|^2 + |T_j|^2 - 2 T_i.T_j, 0) + 1e-8)
    scale_i   = 1 / (1 + mean_j dist[i,j])
    Q2        = (H Wq^T + bq) Wk / sqrt(d)         # bk cancels inside softmax
    E[i,j]    = exp(Q2[i,:] . H[j,:])              # logits bounded ~[-10,10]
    G         = E @ H                              # unnormalized
    out       = ((G / rowsum(E)) Wv^T + bv) * scale @ Wo^T + bo

Sharding: rows of H/T (i dimension) split across 8 cores, 1024 rows each;
H (both orientations) and the small dim x dim weights replicated.

Performance shape (measured on HW): a matmul whose PSUM bank differs from
the previous matmul's issues every N cycles (216 ns at N=512 bf16); a
same-bank successor serializes at ~379 ns. So every inner loop below
alternates PSUM banks between consecutive matmuls:
  - distance phase: pairs of j-chunks accumulate in 2 rotating banks, with
    the |T|^2 row chain for the NEXT group software-pipelined in;
  - attention phase: the k-accumulation of logits for step jt is
    interleaved with the G/rowsum matmuls of step jt-1;
  - small projection chains are emitted pairwise (alternating chunks).
All large matmuls run in bf16 (full PE rate) with f32 PSUM accumulation.
"""

import math
import os
import sys

import numpy as np

for _p in ("/opt/trn_rl_repo", "/root/.axon_site", "/root/.axon_site/_ro/trn_rl_repo"):
    if os.path.isdir(_p) and _p not in sys.path:
        sys.path.append(_p)

import ml_dtypes

import concourse.bass as bass
import concourse.mybir as mybir
import concourse.tile as tile
from concourse import bacc, bass_utils

N = 8192          # total rows
D = 512           # feature dim
NCORES = 8
R = N // NCORES   # rows per core (1024)
P = 128           # partitions
KT = D // P       # 4 contraction tiles
CH = 512          # free-dim chunk (one PSUM bank of f32)
NJC = N // CH     # 16 j-chunks
NJT = N // P      # 64 j-tiles
NIC = R // CH     # 2 i-chunks
NIT = R // P      # 8 i-tiles
JG = 2            # j-chunks per distance group (rotating PSUM banks g0/g1)
NG = NJC // JG    # 8 distance groups
BF = mybir.dt.bfloat16
F32 = mybir.dt.float32
AF = mybir.ActivationFunctionType
ALU = mybir.AluOpType
INV_SQRT_D = 1.0 / math.sqrt(D)

bf16 = ml_dtypes.bfloat16


def _emit(tc, io):
    nc = tc.nc
    from contextlib import ExitStack

    with ExitStack() as ctx:
        const = ctx.enter_context(tc.tile_pool(name="const", bufs=1))
        psum = ctx.enter_context(tc.tile_pool(name="psum", bufs=1, space="PSUM"))
        dram = ctx.enter_context(tc.tile_pool(name="dram", bufs=1, space="DRAM"))
        # attention-phase pools created up front so their SBUF space is
        # carved out early: their first DMAs must not wait on the early
        # pool's release.
        e_pool = ctx.enter_context(tc.tile_pool(name="ep", bufs=6))
        h_pool = ctx.enter_context(tc.tile_pool(name="hp", bufs=8))
        o_pool = ctx.enter_context(tc.tile_pool(name="op", bufs=2))

        # ---- small shared constants ----------------------------------------
        ones_p = const.tile([P, 1], BF, name="onesp")
        nc.vector.memset(ones_p, 1.0)
        ones_f1 = const.tile([1, P], F32, name="onesf1")
        nc.vector.memset(ones_f1, 1.0)
        ones_b1 = const.tile([1, P], BF, name="onesb1")
        nc.vector.memset(ones_b1, 1.0)
        eps_col = const.tile([P, 1], F32, name="epscol")
        nc.vector.memset(eps_col, 1e-8)
        # [128,128] stationary whose first column is ones: full-array config
        # rowsum matmuls (a [1,N] psum output forces a 32-col array-config
        # switch costing ~93 ns on itself and its successor)
        onesw = const.tile([P, P], BF, name="onesw")
        nc.vector.memset(onesw, 0.0)
        nc.vector.memset(onesw[:, 0:1], 1.0)

        # ---- long-lived tensors (written early, read late) -----------------
        Q2T = [const.tile([P, R], BF, name=f"q2t{k}") for k in range(KT)]
        GT = [const.tile([P, R], BF, name=f"gt{d_}") for d_ in range(KT)]
        YT = [const.tile([P, R], BF, name=f"yt{m}") for m in range(KT)]
        SNB = const.tile([P, R], F32, name="snb")
        scl_row = const.tile([1, R], F32, name="sclrow")
        scl_b = const.tile([1, R], BF, name="sclb")
        rs_row = const.tile([1, R], F32, name="rsrow")
        sn_row = const.tile([1, R], F32, name="snrow")

        # ---- early phases (scoped SBUF) ------------------------------------
        with tc.tile_pool(name="early", bufs=1) as early:
            TcT = []
            for k in range(KT):
                tct_t = early.tile([P, R], BF, name=f"tct{k}")
                nc.sync.dma_start(tct_t, io["TcTb"][k * P:(k + 1) * P, :])
                TcT.append(tct_t)
            # K=128 zero-padded aug operands keep the PE in full-array
            # config (a K=2 matmul switches to a 32-row config, ~2x93 ns)
            aug_lhs = early.tile([P, R], BF, name="auglhs")  # r0: -xx_i/2, r1: 1
            nc.vector.memset(aug_lhs, 0.0)
            # ALU writes must start at partition 0; row 1 is filled via DMA.
            for t_ in range(NIT):
                nc.sync.dma_start(aug_lhs[1:2, t_ * P:(t_ + 1) * P], ones_b1)
            dsum = [early.tile([P, NJC], F32, name=f"dsum{it}")
                    for it in range(NIT)]

            with tc.tile_pool(name="sqp", bufs=3) as sq_pool, \
                 tc.tile_pool(name="ttp", bufs=2) as tt_pool, \
                 tc.tile_pool(name="clp", bufs=3) as clamp_pool, \
                 tc.tile_pool(name="dsp", bufs=3) as dist_pool, \
                 tc.tile_pool(name="augp", bufs=2) as aug_pool:

                # -- xx over this core's own rows -> aug_lhs row 0 -----------
                pssc = [psum.tile([1, CH], F32, tag="mm", bufs=3, name="psxxc")
                        for _ in range(NIC)]
                sqcs = [[None] * KT for _ in range(NIC)]
                for ic in range(NIC):
                    for k in range(KT):
                        sqc = sq_pool.tile([P, CH], BF, tag=f"sq{ic}",
                                           name="sqc")
                        nc.vector.tensor_mul(
                            sqc, TcT[k][:, ic * CH:(ic + 1) * CH],
                            TcT[k][:, ic * CH:(ic + 1) * CH])
                        sqcs[ic][k] = sqc
                for k in range(KT):
                    for ic in range(NIC):
                        nc.tensor.matmul(pssc[ic], ones_p, sqcs[ic][k],
                                         start=(k == 0), stop=(k == KT - 1))
                for ic in range(NIC):
                    nc.vector.tensor_scalar(
                        aug_lhs[0:1, ic * CH:(ic + 1) * CH], pssc[ic],
                        -0.5, None, op0=ALU.mult)

                def load_group(jg):
                    tts = [[None] * JG for _ in range(KT)]
                    for jj in range(JG):
                        jc = jg * JG + jj
                        for k in range(KT):
                            tt_t = tt_pool.tile([P, CH], BF, tag=f"tt{k}{jj}",
                                                name=f"ttd{k}")
                            nc.sync.dma_start(
                                tt_t, io["TTb"][k * P:(k + 1) * P,
                                                jc * CH:(jc + 1) * CH])
                            tts[k][jj] = tt_t
                    return tts

                def xx_chain(jg, tts):
                    # squares on DVE (jj=0) and ACT (jj=1); -xx/2 chunks land
                    # in row 1 of the per-group aug tile via SBUF->SBUF DMA.
                    augg = aug_pool.tile([P, JG * CH], BF, tag="augg",
                                         name="augg")
                    nc.vector.memset(augg, 0.0)
                    nc.vector.memset(augg[0:1, :], 1.0)
                    pxx = [psum.tile([1, CH], F32, tag="mm", bufs=3,
                                     name="psxx") for _ in range(JG)]
                    sqs = [[None] * KT for _ in range(JG)]
                    for jj in range(JG):
                        for k in range(KT):
                            sq = sq_pool.tile([P, CH], BF, tag=f"sq{jj}",
                                              name="sq")
                            if jj == 0:
                                nc.vector.tensor_mul(sq, tts[k][jj],
                                                     tts[k][jj])
                            else:
                                nc.scalar.square(sq, tts[k][jj])
                            sqs[jj][k] = sq
                    for k in range(KT):
                        for jj in range(JG):
                            nc.tensor.matmul(pxx[jj], ones_p, sqs[jj][k],
                                             start=(k == 0),
                                             stop=(k == KT - 1))
                    for jj in range(JG):
                        xst = sq_pool.tile([1, CH], BF, tag="xst", bufs=2,
                                           name="xst")
                        nc.vector.tensor_scalar(xst, pxx[jj], -0.5, None,
                                                op0=ALU.mult)
                        nc.sync.dma_start(
                            augg[1:2, jj * CH:(jj + 1) * CH], xst)
                    return augg

                def d2_group(jg, tts, augg):
                    for it in range(NIT):
                        # alternate bank pairs per it so the next iteration
                        # never waits on this one's drains
                        base = 2 * (it % 2)
                        pd = [psum.tile([P, CH], F32, tag=f"g{base + jj}",
                                        name=f"psd{jj}") for jj in range(JG)]
                        for k in range(KT):
                            for jj in range(JG):
                                nc.tensor.matmul(
                                    pd[jj], TcT[k][:, it * P:(it + 1) * P],
                                    tts[k][jj], start=(k == 0), stop=False)
                        for jj in range(JG):
                            nc.tensor.matmul(
                                pd[jj], aug_lhs[:, it * P:(it + 1) * P],
                                augg[:, jj * CH:(jj + 1) * CH],
                                start=False, stop=True)
                        for jj in range(JG):
                            jc = jg * JG + jj
                            t_cl = clamp_pool.tile([P, CH], BF, tag="clamp",
                                                   name="tcl")
                            nc.vector.tensor_scalar(t_cl, pd[jj], -2.0, 0.0,
                                                    op0=ALU.mult, op1=ALU.max)
                            dist_t = dist_pool.tile([P, CH], BF, tag="dist",
                                                    name="distt")
                            nc.scalar.activation(
                                dist_t, t_cl, AF.Sqrt, bias=eps_col,
                                accum_out=dsum[it][:, jc:jc + 1])

                tts_cur = load_group(0)
                augg_cur = xx_chain(0, tts_cur)

                # -- Q chain (independent; overlaps the distance stream) -----
                with tc.tile_pool(name="qpool", bufs=1) as qpool:
                    HcT, WqT, Wk = [], [], []
                    for k in range(KT):
                        hct_t = qpool.tile([P, R], BF, name=f"hct{k}")
                        nc.sync.dma_start(hct_t,
                                          io["HcTb"][k * P:(k + 1) * P, :])
                        HcT.append(hct_t)
                        wqt_t = qpool.tile([P, D], BF, name=f"wqt{k}")
                        nc.sync.dma_start(wqt_t,
                                          io["WqTb"][k * P:(k + 1) * P, :])
                        WqT.append(wqt_t)
                        wk_t = qpool.tile([P, D], BF, name=f"wk{k}")
                        nc.sync.dma_start(wk_t,
                                          io["Wkb"][k * P:(k + 1) * P, :])
                        Wk.append(wk_t)
                    bq_sb = []
                    for m in range(KT):
                        b_t = qpool.tile([P, 1], F32, name=f"bq{m}")
                        nc.sync.dma_start(b_t, io["bqf"][m * P:(m + 1) * P, :])
                        bq_sb.append(b_t)
                    QT = [qpool.tile([P, R], BF, name=f"qt{m}")
                          for m in range(KT)]
                    for m in range(KT):
                        pq = [psum.tile([P, CH], F32, tag="mm", bufs=3,
                                        name="psq") for _ in range(NIC)]
                        for d_ in range(KT):
                            for ic in range(NIC):
                                nc.tensor.matmul(
                                    pq[ic], WqT[d_][:, m * P:(m + 1) * P],
                                    HcT[d_][:, ic * CH:(ic + 1) * CH],
                                    start=(d_ == 0), stop=(d_ == KT - 1))
                        for ic in range(NIC):
                            nc.scalar.activation(
                                QT[m][:, ic * CH:(ic + 1) * CH], pq[ic],
                                AF.Identity, bias=bq_sb[m])
                    for k in range(KT):
                        pq2 = [psum.tile([P, CH], F32, tag="mm", bufs=3,
                                         name="psq2") for _ in range(NIC)]
                        for m in range(KT):
                            for ic in range(NIC):
                                nc.tensor.matmul(
                                    pq2[ic], Wk[m][:, k * P:(k + 1) * P],
                                    QT[m][:, ic * CH:(ic + 1) * CH],
                                    start=(m == 0), stop=(m == KT - 1))
                        for ic in range(NIC):
                            nc.scalar.activation(
                                Q2T[k][:, ic * CH:(ic + 1) * CH], pq2[ic],
                                AF.Copy, scale=INV_SQRT_D)

                # -- distance groups, software pipelined ---------------------
                for jg in range(NG):
                    if jg + 1 < NG:
                        tts_next = load_group(jg + 1)
                        augg_next = xx_chain(jg + 1, tts_next)
                    else:
                        tts_next = augg_next = None
                    d2_group(jg, tts_cur, augg_cur)
                    tts_cur, augg_cur = tts_next, augg_next

            scl_dram = dram.tile([R, 1], F32, name="scldram")
            for it in range(NIT):
                red = early.tile([P, 1], F32, name=f"red{it}")
                nc.vector.reduce_sum(red, dsum[it], axis=mybir.AxisListType.X)
                tmp = early.tile([P, 1], F32, name=f"sctmp{it}")
                nc.vector.tensor_scalar(tmp, red, 1.0 / N, 1.0, op0=ALU.mult,
                                        op1=ALU.add)
                scol = early.tile([P, 1], F32, name=f"scol{it}")
                nc.vector.reciprocal(scol, tmp)
                nc.sync.dma_start(scl_dram[it * P:(it + 1) * P, :], scol)
            nc.sync.dma_start(scl_row,
                              scl_dram.rearrange("(a p) c -> a (p c)", a=1))
            nc.vector.tensor_copy(scl_b, scl_row)

        # ---- tail weights + resident transposed H --------------------------
        wpool = ctx.enter_context(tc.tile_pool(name="wp", bufs=1))
        bv_row = wpool.tile([1, D], BF, name="bvrow")
        nc.sync.dma_start(bv_row, io["bvb"][:, :])
        bo_row = wpool.tile([1, D], BF, name="borow")
        nc.sync.dma_start(bo_row, io["bob"][:, :])
        WvT, WoT = [], []
        for m in range(KT):
            wvt_t = wpool.tile([P, D], BF, name=f"wvt{m}")
            nc.sync.dma_start(wvt_t, io["WvTb"][m * P:(m + 1) * P, :])
            WvT.append(wvt_t)
            wot_t = wpool.tile([P, D], BF, name=f"wot{m}")
            nc.sync.dma_start(wot_t, io["WoTb"][m * P:(m + 1) * P, :])
            WoT.append(wot_t)
        HT = []
        for k in range(KT):
            ht_t = const.tile([P, N], BF, name=f"ht{k}")
            # quarter-row chunks: keep individual HWDGE transfers small so
            # the distance-phase tt stream is not head-of-line blocked
            for q in range(4):
                nc.sync.dma_start(
                    ht_t[:, q * (N // 4):(q + 1) * (N // 4)],
                    io["HTb"][k * P:(k + 1) * P, q * (N // 4):(q + 1) * (N // 4)])
            HT.append(ht_t)

        # ---- attention passes: pipelined logits(jt) | G/rowsum(jt-1) -------
        def attention_pass(ic):
            csl = slice(ic * CH, (ic + 1) * CH)
            g_ps = [psum.tile([P, CH], F32, tag=f"g{d_}", name=f"gps{d_}")
                    for d_ in range(KT)]
            rs_ps = psum.tile([P, CH], F32, tag="rowps", name="rsps")
            # two-deep pipeline: G/rowsum lag the logits by 2 steps so the
            # exp of step jt-2 is long done when its G matmuls issue
            pipe = []  # [(e_t, h_t, jt), ...]
            for jt in range(NJT):
                h_t = h_pool.tile([P, D], BF, tag="h", name="ht_s")
                nc.sync.dma_start(h_t, io["Hb"][jt * P:(jt + 1) * P, :])
                st = psum.tile([P, CH], F32, tag="mm", bufs=3, name="st")
                lag = pipe[0] if len(pipe) == 2 else None
                for k in range(KT):
                    nc.tensor.matmul(st, HT[k][:, jt * P:(jt + 1) * P],
                                     Q2T[k][:, csl],
                                     start=(k == 0), stop=(k == KT - 1))
                    if lag is not None:
                        nc.tensor.matmul(g_ps[k],
                                         lag[1][:, k * P:(k + 1) * P], lag[0],
                                         start=(lag[2] == 0), stop=False)
                if lag is not None:
                    nc.tensor.matmul(rs_ps, onesw, lag[0],
                                     start=(lag[2] == 0), stop=False)
                    pipe.pop(0)
                e_t = e_pool.tile([P, CH], BF, tag="e", name="et")
                nc.scalar.activation(e_t, st, AF.Exp)
                pipe.append((e_t, h_t, jt))
            for (e_t, h_t, jt) in pipe:
                last = jt == NJT - 1
                for k in range(KT):
                    nc.tensor.matmul(g_ps[k], h_t[:, k * P:(k + 1) * P], e_t,
                                     start=False, stop=last)
                nc.tensor.matmul(rs_ps, onesw, e_t, start=False, stop=last)
            # drain accumulators promptly so the next pass can claim the banks
            for d_ in range(KT):
                nc.scalar.activation(GT[d_][:, csl], g_ps[d_], AF.Copy)
            nc.vector.tensor_copy(rs_row[0:1, csl], rs_ps[0:1, :])

        def tail(ic):
            csl = slice(ic * CH, (ic + 1) * CH)
            nc.vector.reciprocal(sn_row[0:1, csl], rs_row[0:1, csl])
            nc.vector.tensor_mul(sn_row[0:1, csl], sn_row[0:1, csl],
                                 scl_row[0:1, csl])
            ps_snb = psum.tile([P, CH], F32, tag="mm", bufs=3, name="pssnb")
            nc.tensor.matmul(ps_snb, ones_f1, sn_row[0:1, csl],
                             start=True, stop=True)
            nc.vector.tensor_copy(SNB[:, csl], ps_snb)
            for d_ in range(KT):
                nc.vector.tensor_mul(GT[d_][:, csl], GT[d_][:, csl],
                                     SNB[:, csl])
            # Y^T = Wv Gn^T + (bv x scale): two m-chains in flight
            for m0 in range(0, KT, 2):
                py = [psum.tile([P, CH], F32, tag="mm", bufs=3, name="psy")
                      for _ in range(2)]
                for d_ in range(KT):
                    for u in range(2):
                        m = m0 + u
                        nc.tensor.matmul(py[u], WvT[d_][:, m * P:(m + 1) * P],
                                         GT[d_][:, csl],
                                         start=(d_ == 0), stop=False)
                for u in range(2):
                    m = m0 + u
                    nc.tensor.matmul(py[u], bv_row[0:1, m * P:(m + 1) * P],
                                     scl_b[0:1, csl], start=False, stop=True)
                for u in range(2):
                    m = m0 + u
                    nc.scalar.activation(YT[m][:, csl], py[u], AF.Copy)
            # out = Y Wo^T + bo for this chunk's 4 i-tiles, chains in pairs
            for it0 in range(ic * 4, (ic + 1) * 4, 2):
                po = [psum.tile([P, CH], F32, tag="mm", bufs=3, name="pso")
                      for _ in range(2)]
                for m in range(KT):
                    for u in range(2):
                        it = it0 + u
                        nc.tensor.matmul(po[u], YT[m][:, it * P:(it + 1) * P],
                                         WoT[m], start=(m == 0), stop=False)
                for u in range(2):
                    nc.tensor.matmul(po[u], ones_b1, bo_row,
                                     start=False, stop=True)
                for u in range(2):
                    it = it0 + u
                    o_t = o_pool.tile([P, D], F32, tag="o", name="ot")
                    nc.scalar.activation(o_t, po[u], AF.Copy)
                    nc.sync.dma_start(io["OUT"][it * P:(it + 1) * P, :], o_t)

        attention_pass(0)
        attention_pass(1)
        tail(0)
        tail(1)


_NC_CACHE = None


def _build():
    global _NC_CACHE
    if _NC_CACHE is not None:
        return _NC_CACHE
    nc = bacc.Bacc("TRN2", target_bir_lowering=False, debug=False,
                   enable_asserts=False, num_devices=NCORES)
    io = {
        "HTb": nc.dram_tensor("HTb", [D, N], BF, kind="ExternalInput").ap(),
        "Hb": nc.dram_tensor("Hb", [N, D], BF, kind="ExternalInput").ap(),
        "TTb": nc.dram_tensor("TTb", [D, N], BF, kind="ExternalInput").ap(),
        "TcTb": nc.dram_tensor("TcTb", [D, R], BF, kind="ExternalInput").ap(),
        "HcTb": nc.dram_tensor("HcTb", [D, R], BF, kind="ExternalInput").ap(),
        "WqTb": nc.dram_tensor("WqTb", [D, D], BF, kind="ExternalInput").ap(),
        "Wkb": nc.dram_tensor("Wkb", [D, D], BF, kind="ExternalInput").ap(),
        "WvTb": nc.dram_tensor("WvTb", [D, D], BF, kind="ExternalInput").ap(),
        "WoTb": nc.dram_tensor("WoTb", [D, D], BF, kind="ExternalInput").ap(),
        "bqf": nc.dram_tensor("bqf", [D, 1], F32, kind="ExternalInput").ap(),
        "bvb": nc.dram_tensor("bvb", [1, D], BF, kind="ExternalInput").ap(),
        "bob": nc.dram_tensor("bob", [1, D], BF, kind="ExternalInput").ap(),
        "OUT": nc.dram_tensor("OUT", [R, D], F32, kind="ExternalOutput").ap(),
    }
    with tile.TileContext(nc) as tc:
        _emit(tc, io)
    nc.compile()
    _NC_CACHE = nc
    return nc


LAST_RESULTS = None


def kernel(H, T, Wq, bq, Wk, bk, Wv, bv, Wo, bo):
    global LAST_RESULTS
    H = np.ascontiguousarray(np.asarray(H, np.float32))
    T = np.ascontiguousarray(np.asarray(T, np.float32))

    HTb = np.ascontiguousarray(H.T).astype(bf16)
    Hb = H.astype(bf16)
    TTb = np.ascontiguousarray(T.T).astype(bf16)
    shared = {
        "HTb": HTb,
        "Hb": Hb,
        "TTb": TTb,
        "WqTb": np.ascontiguousarray(np.asarray(Wq, np.float32).T).astype(bf16),
        "Wkb": np.ascontiguousarray(np.asarray(Wk, np.float32)).astype(bf16),
        "WvTb": np.ascontiguousarray(np.asarray(Wv, np.float32).T).astype(bf16),
        "WoTb": np.ascontiguousarray(np.asarray(Wo, np.float32).T).astype(bf16),
        "bqf": np.asarray(bq, np.float32).reshape(D, 1).copy(),
        "bvb": np.asarray(bv, np.float32).reshape(1, D).astype(bf16),
        "bob": np.asarray(bo, np.float32).reshape(1, D).astype(bf16),
    }
    in_maps = []
    for c in range(NCORES):
        m = dict(shared)
        m["TcTb"] = np.ascontiguousarray(TTb[:, c * R:(c + 1) * R])
        m["HcTb"] = np.ascontiguousarray(HTb[:, c * R:(c + 1) * R])
        in_maps.append(m)

    nc = _build()
    res = bass_utils.run_bass_kernel_spmd(nc, in_maps, core_ids=list(range(NCORES)))
    LAST_RESULTS = res
    out = np.concatenate([res.results[c]["OUT"] for c in range(NCORES)], axis=0)
    return np.ascontiguousarray(out.astype(np.float32))



# revision 7
# speedup vs baseline: 1.2444x; 1.2444x over previous
"""Trainium2 Bass kernel for a causal-attention-like module (fp8 DoubleRow).

Math (reassociated from the reference nn.Module):
    dist[i,j] = sqrt(max(|T_i|^2 + |T_j|^2 - 2 T_i.T_j, 0) + 1e-8)
    scale_i   = 1 / (1 + mean_j dist[i,j])
    Q2        = (H Wq^T + bq) Wk / sqrt(d)         # bk cancels inside softmax
    E[i,j]    = exp(Q2[i,:] . H[j,:] - 3)          # -3 shift cancels in rownorm
    G         = E @ H                              # unnormalized
    out       = ((G / rowsum(E)) Wv^T + bv) * scale @ Wo^T + bo

Sharding: rows of H/T (i dimension) split across 8 cores, 1024 rows each;
H (both orientations) and the small dim x dim weights replicated.

Performance shape: the three N*R*D matmuls (distance, logits, E@H) plus the
rowsum run as fp8e4 DoubleRow matmuls (two 128-deep contraction planes per
instruction, PE double-pumped). Operand layout is [128, plane, X] so a
[:, 2u:2u+2, x] slice gives the [K,2,M] shape DoubleRow expects. PSUM banks
alternate between consecutive matmuls (same-bank successors serialize).
fp8 accuracy safeguards:
  - xx augmentation is centered at 512 and carried in two rows (value +
    residual) so e4m3's 3-bit mantissa does not perturb the distance scale;
  - Q2 is scaled by 16 into fp8's normal range; exp() applies scale 1/16
    and bias -3 so E stays well under the TRN fp8e4 max of 240;
  - the small dim x dim projection chains stay bf16.
"""

import math
import os
import sys

import numpy as np

for _p in ("/opt/trn_rl_repo", "/root/.axon_site", "/root/.axon_site/_ro/trn_rl_repo"):
    if os.path.isdir(_p) and _p not in sys.path:
        sys.path.append(_p)

import ml_dtypes

import concourse.bass as bass
import concourse.mybir as mybir
import concourse.tile as tile
from concourse import bacc, bass_utils

N = 8192          # total rows
D = 512           # feature dim
NCORES = 8
R = N // NCORES   # rows per core (1024)
P = 128           # partitions
KT = D // P       # 4 contraction planes
CH = 512          # free-dim chunk (one PSUM bank of f32)
NJC = N // CH     # 16 j-chunks
NJT = N // P      # 64 j-tiles
NPAIR = NJT // 2  # 32 j-tile pairs (DoubleRow granularity)
NIC = R // CH     # 2 i-chunks
NIT = R // P      # 8 i-tiles
JG = 2            # j-chunks per distance group (rotating PSUM banks)
NG = NJC // JG    # 8 distance groups
BF = mybir.dt.bfloat16
F32 = mybir.dt.float32
F8 = mybir.dt.float8e4
DR = mybir.MatmulPerfMode.DoubleRow
AF = mybir.ActivationFunctionType
ALU = mybir.AluOpType
Q2SCALE = 16.0 / math.sqrt(D)   # Q2 stored as 16*Q2_true in fp8
EXPSCALE = 1.0 / 16.0
EXPBIAS = -4.75

bf16 = ml_dtypes.bfloat16
f8e4 = ml_dtypes.float8_e4m3


def _emit(tc, io):
    nc = tc.nc
    from contextlib import ExitStack

    with ExitStack() as ctx:
        const = ctx.enter_context(tc.tile_pool(name="const", bufs=1))
        psum = ctx.enter_context(tc.tile_pool(name="psum", bufs=1, space="PSUM"))
        # attention-phase pools created up front so their SBUF space is
        # carved out early: their first DMAs must not wait on the early
        # pool's release.
        e_pool = ctx.enter_context(tc.tile_pool(name="ep", bufs=4))
        h_pool = ctx.enter_context(tc.tile_pool(name="hp", bufs=4))
        o_pool = ctx.enter_context(tc.tile_pool(name="op", bufs=2))

        # ---- small shared constants ----------------------------------------
        ones_f1 = const.tile([1, P], F32, name="onesf1")
        nc.vector.memset(ones_f1, 1.0)
        ones_b1 = const.tile([1, P], BF, name="onesb1")
        nc.vector.memset(ones_b1, 1.0)
        eps_col = const.tile([P, 1], F32, name="epscol")
        nc.vector.memset(eps_col, 1e-8)
        expb_col = const.tile([P, 1], F32, name="expbcol")
        nc.vector.memset(expb_col, EXPBIAS)
        # [128,2,128] fp8 stationary whose first column is ones in both
        # planes: DoubleRow rowsum over a pair of 128-row tiles.
        onesw2 = const.tile([P, 2, P], F8, name="onesw2")
        nc.vector.memset(onesw2, 0.0)
        nc.vector.memset(onesw2[:, :, 0:1], 1.0)
        cst_1r = const.tile([1, R], F8, name="cst1r")
        nc.vector.memset(cst_1r, 1.0)
        cst_m4 = const.tile([1, R], F8, name="cstm4")
        nc.vector.memset(cst_m4, -4.0)
        cst_128 = const.tile([1, JG * CH], F8, name="cst128")
        nc.vector.memset(cst_128, 128.0)

        # ---- long-lived tensors (written early, read late) -----------------
        q2f8 = const.tile([P, KT, R], F8, name="q2f8")
        GT = [const.tile([P, R], BF, name=f"gt{d_}") for d_ in range(KT)]
        YT = [const.tile([P, R], BF, name=f"yt{m}") for m in range(KT)]
        SNB = const.tile([P, R], F32, name="snb")
        scl_row = const.tile([1, R], F32, name="sclrow")
        scl_b = const.tile([1, R], BF, name="sclb")
        rs_row = const.tile([1, R], F32, name="rsrow")
        sn_row = const.tile([1, R], F32, name="snrow")

        # ---- early phases (scoped SBUF) ------------------------------------
        with tc.tile_pool(name="early", bufs=1) as early:
            tct = early.tile([P, KT, R], F8, name="tct")
            for k in range(KT):
                nc.sync.dma_start(tct[:, k:k + 1, :],
                                  io["TcTf8"][k * P:(k + 1) * P, :])
            # aug operand, 128-deep zero-padded plane0 + zero plane1 so the
            # aug matmul is a normal full-array DoubleRow instruction.
            # plane0 rows: r0=q_i, r1=res_i (pair with moving ones),
            # r2=1, r3=1 (pair with moving q_j, res_j), r4=-4 (pairs with
            # moving 128 -> -512 constant); q+res = -(xx-512)/2.
            aug_lhs = early.tile([P, 2, R], F8, name="auglhs")
            nc.vector.memset(aug_lhs, 0.0)
            nc.sync.dma_start(aug_lhs[2:3, 0:1, :], cst_1r)
            nc.sync.dma_start(aug_lhs[3:4, 0:1, :], cst_1r)
            nc.sync.dma_start(aug_lhs[4:5, 0:1, :], cst_m4)
            dsum = [early.tile([P, NJC], F32, name=f"dsum{it}")
                    for it in range(NIT)]

            with tc.tile_pool(name="sqp", bufs=3) as sq_pool, \
                 tc.tile_pool(name="ttp", bufs=2) as tt_pool, \
                 tc.tile_pool(name="clp", bufs=3) as clamp_pool, \
                 tc.tile_pool(name="dsp", bufs=3) as dist_pool, \
                 tc.tile_pool(name="augp", bufs=2) as aug_pool:

                # -- xx over this core's own rows -> aug_lhs rows 0/1 --------
                pssc = [psum.tile([P, CH], F32, tag="mm", bufs=3, name="psxxc")
                        for _ in range(NIC)]
                sqcs = [[None] * 2 for _ in range(NIC)]
                for ic in range(NIC):
                    for u in range(2):
                        sqc = sq_pool.tile([P, 2, CH], F8, tag=f"sq{ic}{u}",
                                           name="sqc")
                        for pl in range(2):
                            k = 2 * u + pl
                            eng = nc.vector if pl == 0 else nc.gpsimd
                            eng.tensor_mul(
                                sqc[:, pl:pl + 1, :],
                                tct[:, k:k + 1, ic * CH:(ic + 1) * CH],
                                tct[:, k:k + 1, ic * CH:(ic + 1) * CH])
                        sqcs[ic][u] = sqc
                for u in range(2):
                    for ic in range(NIC):
                        nc.tensor.matmul(pssc[ic], onesw2, sqcs[ic][u],
                                         start=(u == 0), stop=(u == 1),
                                         perf_mode=DR)
                for ic in range(NIC):
                    csl = slice(ic * CH, (ic + 1) * CH)
                    tv = sq_pool.tile([1, CH], F32, tag="tv", bufs=2,
                                      name="tvc")
                    nc.vector.tensor_scalar(tv, pssc[ic][0:1, :], -0.5, 256.0,
                                            op0=ALU.mult, op1=ALU.add)
                    xq = sq_pool.tile([1, CH], F8, tag="xqc", bufs=2,
                                      name="xqc")
                    nc.vector.tensor_copy(xq, tv)
                    xr = sq_pool.tile([1, CH], F8, tag="xrc", bufs=2,
                                      name="xrc")
                    nc.vector.tensor_sub(xr, tv, xq)
                    nc.sync.dma_start(aug_lhs[0:1, 0:1, csl], xq)
                    nc.sync.dma_start(aug_lhs[1:2, 0:1, csl], xr)

                def load_group(jg):
                    tts = []
                    for jj in range(JG):
                        jc = jg * JG + jj
                        tt_t = tt_pool.tile([P, KT, CH], F8, tag=f"tt{jj}",
                                            name="ttd")
                        for k in range(KT):
                            nc.sync.dma_start(
                                tt_t[:, k:k + 1, :],
                                io["TTf8"][k * P:(k + 1) * P,
                                           jc * CH:(jc + 1) * CH])
                        tts.append(tt_t)
                    return tts

                def xx_chain(jg, tts):
                    # squares on DVE+GPSIMD; xx row via DoubleRow ones-matmul;
                    # value+residual rows land in augg plane0 via SBUF DMA.
                    augg = aug_pool.tile([P, 2, JG * CH], F8, tag="augg",
                                         name="augg")
                    nc.vector.memset(augg, 0.0)
                    nc.vector.memset(augg[0:1, 0:1, :], 1.0)
                    nc.sync.dma_start(augg[1:2, 0:1, :], cst_1r)
                    nc.sync.dma_start(augg[4:5, 0:1, :], cst_128)
                    pxx = [psum.tile([P, CH], F32, tag="mm", bufs=3,
                                     name="psxx") for _ in range(JG)]
                    sqs = [[None] * 2 for _ in range(JG)]
                    for jj in range(JG):
                        for u in range(2):
                            sq = sq_pool.tile([P, 2, CH], F8, tag=f"sq{jj}{u}",
                                              name="sq")
                            for pl in range(2):
                                k = 2 * u + pl
                                eng = nc.vector if pl == 0 else nc.gpsimd
                                eng.tensor_mul(sq[:, pl:pl + 1, :],
                                               tts[jj][:, k:k + 1, :],
                                               tts[jj][:, k:k + 1, :])
                            sqs[jj][u] = sq
                    for u in range(2):
                        for jj in range(JG):
                            nc.tensor.matmul(pxx[jj], onesw2, sqs[jj][u],
                                             start=(u == 0), stop=(u == 1),
                                             perf_mode=DR)
                    for jj in range(JG):
                        tv = sq_pool.tile([1, CH], F32, tag="tvj", bufs=2,
                                          name="tvj")
                        nc.vector.tensor_scalar(tv, pxx[jj][0:1, :], -0.5,
                                                256.0, op0=ALU.mult,
                                                op1=ALU.add)
                        xq = sq_pool.tile([1, CH], F8, tag="xq", bufs=2,
                                          name="xq")
                        nc.vector.tensor_copy(xq, tv)
                        xr = sq_pool.tile([1, CH], F8, tag="xr", bufs=2,
                                          name="xr")
                        nc.vector.tensor_sub(xr, tv, xq)
                        nc.sync.dma_start(
                            augg[2:3, 0:1, jj * CH:(jj + 1) * CH], xq)
                        nc.sync.dma_start(
                            augg[3:4, 0:1, jj * CH:(jj + 1) * CH], xr)
                    return augg

                def d2_group(jg, tts, augg):
                    for it in range(NIT):
                        # alternate bank pairs per it so the next iteration
                        # never waits on this one's drains
                        base = 2 * (it % 2)
                        pd = [psum.tile([P, CH], F32, tag=f"g{base + jj}",
                                        name=f"psd{jj}") for jj in range(JG)]
                        for u in range(2):
                            for jj in range(JG):
                                nc.tensor.matmul(
                                    pd[jj],
                                    tct[:, 2 * u:2 * u + 2,
                                        it * P:(it + 1) * P],
                                    tts[jj][:, 2 * u:2 * u + 2, :],
                                    start=(u == 0), stop=False, perf_mode=DR)
                        for jj in range(JG):
                            nc.tensor.matmul(
                                pd[jj], aug_lhs[:, :, it * P:(it + 1) * P],
                                augg[:, :, jj * CH:(jj + 1) * CH],
                                start=False, stop=True, perf_mode=DR)
                        for jj in range(JG):
                            jc = jg * JG + jj
                            t_cl = clamp_pool.tile([P, CH], BF, tag="clamp",
                                                   name="tcl")
                            nc.vector.tensor_scalar(t_cl, pd[jj], -2.0, 0.0,
                                                    op0=ALU.mult, op1=ALU.max)
                            dist_t = dist_pool.tile([P, CH], BF, tag="dist",
                                                    name="distt")
                            nc.scalar.activation(
                                dist_t, t_cl, AF.Sqrt, bias=eps_col,
                                accum_out=dsum[it][:, jc:jc + 1])

                tts_cur = load_group(0)
                augg_cur = xx_chain(0, tts_cur)

                # -- Q chain (independent; overlaps the distance stream) -----
                with tc.tile_pool(name="qpool", bufs=1) as qpool:
                    HcT, WqT, Wk = [], [], []
                    for k in range(KT):
                        hct_t = qpool.tile([P, R], BF, name=f"hct{k}")
                        nc.sync.dma_start(hct_t,
                                          io["HcTb"][k * P:(k + 1) * P, :])
                        HcT.append(hct_t)
                        wqt_t = qpool.tile([P, D], BF, name=f"wqt{k}")
                        nc.sync.dma_start(wqt_t,
                                          io["WqTb"][k * P:(k + 1) * P, :])
                        WqT.append(wqt_t)
                        wk_t = qpool.tile([P, D], BF, name=f"wk{k}")
                        nc.sync.dma_start(wk_t,
                                          io["Wkb"][k * P:(k + 1) * P, :])
                        Wk.append(wk_t)
                    bq_sb = []
                    for m in range(KT):
                        b_t = qpool.tile([P, 1], F32, name=f"bq{m}")
                        nc.sync.dma_start(b_t, io["bqf"][m * P:(m + 1) * P, :])
                        bq_sb.append(b_t)
                    QT = [qpool.tile([P, R], BF, name=f"qt{m}")
                          for m in range(KT)]
                    for m in range(KT):
                        pq = [psum.tile([P, CH], F32, tag="mm", bufs=3,
                                        name="psq") for _ in range(NIC)]
                        for d_ in range(KT):
                            for ic in range(NIC):
                                nc.tensor.matmul(
                                    pq[ic], WqT[d_][:, m * P:(m + 1) * P],
                                    HcT[d_][:, ic * CH:(ic + 1) * CH],
                                    start=(d_ == 0), stop=(d_ == KT - 1))
                        for ic in range(NIC):
                            nc.scalar.activation(
                                QT[m][:, ic * CH:(ic + 1) * CH], pq[ic],
                                AF.Identity, bias=bq_sb[m])
                    for k in range(KT):
                        pq2 = [psum.tile([P, CH], F32, tag="mm", bufs=3,
                                         name="psq2") for _ in range(NIC)]
                        for m in range(KT):
                            for ic in range(NIC):
                                nc.tensor.matmul(
                                    pq2[ic], Wk[m][:, k * P:(k + 1) * P],
                                    QT[m][:, ic * CH:(ic + 1) * CH],
                                    start=(m == 0), stop=(m == KT - 1))
                        for ic in range(NIC):
                            nc.scalar.activation(
                                q2f8[:, k:k + 1, ic * CH:(ic + 1) * CH],
                                pq2[ic], AF.Copy, scale=Q2SCALE)

                # -- distance groups, software pipelined ---------------------
                for jg in range(NG):
                    if jg + 1 < NG:
                        tts_next = load_group(jg + 1)
                        augg_next = xx_chain(jg + 1, tts_next)
                    else:
                        tts_next = augg_next = None
                    d2_group(jg, tts_cur, augg_cur)
                    tts_cur, augg_cur = tts_next, augg_next

            with tc.tile_pool(name="scl", bufs=1, space="DRAM") as dram:
                scl_dram = dram.tile([R, 1], F32, name="scldram")
                for it in range(NIT):
                    red = early.tile([P, 1], F32, name=f"red{it}")
                    nc.vector.reduce_sum(red, dsum[it],
                                         axis=mybir.AxisListType.X)
                    tmp = early.tile([P, 1], F32, name=f"sctmp{it}")
                    nc.vector.tensor_scalar(tmp, red, 1.0 / N, 1.0,
                                            op0=ALU.mult, op1=ALU.add)
                    scol = early.tile([P, 1], F32, name=f"scol{it}")
                    nc.vector.reciprocal(scol, tmp)
                    nc.sync.dma_start(scl_dram[it * P:(it + 1) * P, :], scol)
                nc.sync.dma_start(
                    scl_row, scl_dram.rearrange("(a p) c -> a (p c)", a=1))
                nc.vector.tensor_copy(scl_b, scl_row)

        # ---- tail weights + resident transposed H (fp8) --------------------
        wpool = ctx.enter_context(tc.tile_pool(name="wp", bufs=1))
        bv_row = wpool.tile([1, D], BF, name="bvrow")
        nc.sync.dma_start(bv_row, io["bvb"][:, :])
        bo_row = wpool.tile([1, D], BF, name="borow")
        nc.sync.dma_start(bo_row, io["bob"][:, :])
        WvT, WoT = [], []
        for m in range(KT):
            wvt_t = wpool.tile([P, D], BF, name=f"wvt{m}")
            nc.sync.dma_start(wvt_t, io["WvTb"][m * P:(m + 1) * P, :])
            WvT.append(wvt_t)
            wot_t = wpool.tile([P, D], BF, name=f"wot{m}")
            nc.sync.dma_start(wot_t, io["WoTb"][m * P:(m + 1) * P, :])
            WoT.append(wot_t)
        ht8 = wpool.tile([P, KT, N], F8, name="ht8")
        # quarter-row chunks: keep individual HWDGE transfers small so
        # the distance-phase tt stream is not head-of-line blocked
        for k in range(KT):
            for q in range(4):
                nc.sync.dma_start(
                    ht8[:, k:k + 1, q * (N // 4):(q + 1) * (N // 4)],
                    io["HTf8"][k * P:(k + 1) * P,
                               q * (N // 4):(q + 1) * (N // 4)])

        # ---- attention passes: pipelined logits(pair s) | G/rowsum(s-2) ----
        def attention_pass(ic):
            csl = slice(ic * CH, (ic + 1) * CH)
            g_ps = [psum.tile([P, CH], F32, tag=f"g{d_}", name=f"gps{d_}")
                    for d_ in range(KT)]
            rs_ps = psum.tile([P, CH], F32, tag="rowps", name="rsps")
            # two-deep pair pipeline: G/rowsum lag the logits by 2 pairs so
            # the exp of pair s-2 is long done when its G matmuls issue
            pipe = []  # [(e2_t, h2_t, s), ...]

            def g_mm(lag, k, stop=False):
                nc.tensor.matmul(g_ps[k], lag[1][:, :, k * P:(k + 1) * P],
                                 lag[0], start=(lag[2] == 0), stop=stop,
                                 perf_mode=DR)

            for s in range(NPAIR):
                h2_t = h_pool.tile([P, 2, D], F8, tag="h", name="h2t")
                nc.sync.dma_start(h2_t[:, 0:1, :],
                                  io["Hf8"][(2 * s) * P:(2 * s + 1) * P, :])
                nc.sync.dma_start(h2_t[:, 1:2, :],
                                  io["Hf8"][(2 * s + 1) * P:(2 * s + 2) * P, :])
                e2_t = e_pool.tile([P, 2, CH], F8, tag="e", name="e2t")
                st_a = psum.tile([P, CH], F32, tag="mm", bufs=3, name="sta")
                st_b = psum.tile([P, CH], F32, tag="mm", bufs=3, name="stb")
                lag = pipe[0] if len(pipe) == 2 else None
                nc.tensor.matmul(st_a, ht8[:, 0:2, (2 * s) * P:(2 * s + 1) * P],
                                 q2f8[:, 0:2, csl], start=True, stop=False,
                                 perf_mode=DR)
                if lag is not None:
                    g_mm(lag, 0)
                nc.tensor.matmul(st_a, ht8[:, 2:4, (2 * s) * P:(2 * s + 1) * P],
                                 q2f8[:, 2:4, csl], start=False, stop=True,
                                 perf_mode=DR)
                if lag is not None:
                    g_mm(lag, 1)
                nc.scalar.activation(e2_t[:, 0:1, :], st_a, AF.Exp,
                                     scale=EXPSCALE, bias=expb_col)
                nc.tensor.matmul(st_b,
                                 ht8[:, 0:2, (2 * s + 1) * P:(2 * s + 2) * P],
                                 q2f8[:, 0:2, csl], start=True, stop=False,
                                 perf_mode=DR)
                if lag is not None:
                    g_mm(lag, 2)
                nc.tensor.matmul(st_b,
                                 ht8[:, 2:4, (2 * s + 1) * P:(2 * s + 2) * P],
                                 q2f8[:, 2:4, csl], start=False, stop=True,
                                 perf_mode=DR)
                if lag is not None:
                    g_mm(lag, 3)
                    nc.tensor.matmul(rs_ps, onesw2, lag[0],
                                     start=(lag[2] == 0), stop=False,
                                     perf_mode=DR)
                    pipe.pop(0)
                nc.scalar.activation(e2_t[:, 1:2, :], st_b, AF.Exp,
                                     scale=EXPSCALE, bias=expb_col)
                pipe.append((e2_t, h2_t, s))
            for (e2_t, h2_t, s) in pipe:
                last = s == NPAIR - 1
                for k in range(KT):
                    nc.tensor.matmul(g_ps[k], h2_t[:, :, k * P:(k + 1) * P],
                                     e2_t, start=(s == 0), stop=last,
                                     perf_mode=DR)
                nc.tensor.matmul(rs_ps, onesw2, e2_t, start=(s == 0),
                                 stop=last, perf_mode=DR)
            # drain accumulators promptly so the next pass can claim the banks
            for d_ in range(KT):
                nc.scalar.activation(GT[d_][:, csl], g_ps[d_], AF.Copy)
            nc.vector.tensor_copy(rs_row[0:1, csl], rs_ps[0:1, :])

        def tail(ic):
            csl = slice(ic * CH, (ic + 1) * CH)
            nc.vector.reciprocal(sn_row[0:1, csl], rs_row[0:1, csl])
            nc.vector.tensor_mul(sn_row[0:1, csl], sn_row[0:1, csl],
                                 scl_row[0:1, csl])
            ps_snb = psum.tile([P, CH], F32, tag="mm", bufs=3, name="pssnb")
            nc.tensor.matmul(ps_snb, ones_f1, sn_row[0:1, csl],
                             start=True, stop=True)
            nc.vector.tensor_copy(SNB[:, csl], ps_snb)
            for d_ in range(KT):
                nc.vector.tensor_mul(GT[d_][:, csl], GT[d_][:, csl],
                                     SNB[:, csl])
            # Y^T = Wv Gn^T + (bv x scale): two m-chains in flight
            for m0 in range(0, KT, 2):
                py = [psum.tile([P, CH], F32, tag="mm", bufs=3, name="psy")
                      for _ in range(2)]
                for d_ in range(KT):
                    for u in range(2):
                        m = m0 + u
                        nc.tensor.matmul(py[u], WvT[d_][:, m * P:(m + 1) * P],
                                         GT[d_][:, csl],
                                         start=(d_ == 0), stop=False)
                for u in range(2):
                    m = m0 + u
                    nc.tensor.matmul(py[u], bv_row[0:1, m * P:(m + 1) * P],
                                     scl_b[0:1, csl], start=False, stop=True)
                for u in range(2):
                    m = m0 + u
                    nc.scalar.activation(YT[m][:, csl], py[u], AF.Copy)
            # out = Y Wo^T + bo for this chunk's 4 i-tiles, chains in pairs
            for it0 in range(ic * 4, (ic + 1) * 4, 2):
                po = [psum.tile([P, CH], F32, tag="mm", bufs=3, name="pso")
                      for _ in range(2)]
                for m in range(KT):
                    for u in range(2):
                        it = it0 + u
                        nc.tensor.matmul(po[u], YT[m][:, it * P:(it + 1) * P],
                                         WoT[m], start=(m == 0), stop=False)
                for u in range(2):
                    nc.tensor.matmul(po[u], ones_b1, bo_row,
                                     start=False, stop=True)
                for u in range(2):
                    it = it0 + u
                    o_t = o_pool.tile([P, D], F32, tag="o", name="ot")
                    nc.scalar.activation(o_t, po[u], AF.Copy)
                    nc.sync.dma_start(io["OUT"][it * P:(it + 1) * P, :], o_t)

        attention_pass(0)
        attention_pass(1)
        tail(0)
        tail(1)


_NC_CACHE = None


def _build():
    global _NC_CACHE
    if _NC_CACHE is not None:
        return _NC_CACHE
    nc = bacc.Bacc("TRN2", target_bir_lowering=False, debug=False,
                   enable_asserts=False, num_devices=NCORES)
    io = {
        "HTf8": nc.dram_tensor("HTf8", [D, N], F8, kind="ExternalInput").ap(),
        "Hf8": nc.dram_tensor("Hf8", [N, D], F8, kind="ExternalInput").ap(),
        "TTf8": nc.dram_tensor("TTf8", [D, N], F8, kind="ExternalInput").ap(),
        "TcTf8": nc.dram_tensor("TcTf8", [D, R], F8,
                                kind="ExternalInput").ap(),
        "HcTb": nc.dram_tensor("HcTb", [D, R], BF, kind="ExternalInput").ap(),
        "WqTb": nc.dram_tensor("WqTb", [D, D], BF, kind="ExternalInput").ap(),
        "Wkb": nc.dram_tensor("Wkb", [D, D], BF, kind="ExternalInput").ap(),
        "WvTb": nc.dram_tensor("WvTb", [D, D], BF, kind="ExternalInput").ap(),
        "WoTb": nc.dram_tensor("WoTb", [D, D], BF, kind="ExternalInput").ap(),
        "bqf": nc.dram_tensor("bqf", [D, 1], F32, kind="ExternalInput").ap(),
        "bvb": nc.dram_tensor("bvb", [1, D], BF, kind="ExternalInput").ap(),
        "bob": nc.dram_tensor("bob", [1, D], BF, kind="ExternalInput").ap(),
        "OUT": nc.dram_tensor("OUT", [R, D], F32, kind="ExternalOutput").ap(),
    }
    with tile.TileContext(nc) as tc:
        _emit(tc, io)
    nc.compile()
    _NC_CACHE = nc
    return nc


LAST_RESULTS = None


def _to_f8(a):
    return np.clip(a, -240.0, 240.0).astype(f8e4)


def kernel(H, T, Wq, bq, Wk, bk, Wv, bv, Wo, bo):
    global LAST_RESULTS
    H = np.ascontiguousarray(np.asarray(H, np.float32))
    T = np.ascontiguousarray(np.asarray(T, np.float32))

    HT = np.ascontiguousarray(H.T)
    TT = np.ascontiguousarray(T.T)
    HTb = HT.astype(bf16)
    shared = {
        "HTf8": _to_f8(HT),
        "Hf8": _to_f8(H),
        "TTf8": _to_f8(TT),
        "WqTb": np.ascontiguousarray(np.asarray(Wq, np.float32).T).astype(bf16),
        "Wkb": np.ascontiguousarray(np.asarray(Wk, np.float32)).astype(bf16),
        "WvTb": np.ascontiguousarray(np.asarray(Wv, np.float32).T).astype(bf16),
        "WoTb": np.ascontiguousarray(np.asarray(Wo, np.float32).T).astype(bf16),
        "bqf": np.asarray(bq, np.float32).reshape(D, 1).copy(),
        "bvb": np.asarray(bv, np.float32).reshape(1, D).astype(bf16),
        "bob": np.asarray(bo, np.float32).reshape(1, D).astype(bf16),
    }
    in_maps = []
    for c in range(NCORES):
        m = dict(shared)
        m["TcTf8"] = np.ascontiguousarray(shared["TTf8"][:, c * R:(c + 1) * R])
        m["HcTb"] = np.ascontiguousarray(HTb[:, c * R:(c + 1) * R])
        in_maps.append(m)

    nc = _build()
    res = bass_utils.run_bass_kernel_spmd(nc, in_maps,
                                          core_ids=list(range(NCORES)))
    LAST_RESULTS = res
    out = np.concatenate([res.results[c]["OUT"] for c in range(NCORES)],
                         axis=0)
    return np.ascontiguousarray(out.astype(np.float32))


# revision 10
# speedup vs baseline: 1.4893x; 1.1968x over previous
"""Trainium2 Bass kernel for a causal-attention-like module (fp8 DoubleRow).

Math (reassociated from the reference nn.Module):
    dist[i,j] = sqrt(max(|T_i|^2 + |T_j|^2 - 2 T_i.T_j, 0) + 1e-8)
    scale_i   = 1 / (1 + mean_j dist[i,j])
    Q2        = (H Wq^T + bq) Wk / sqrt(d)         # bk cancels inside softmax
    E[i,j]    = exp(Q2[i,:] . H[j,:] - 3)          # -3 shift cancels in rownorm
    G         = E @ H                              # unnormalized
    out       = ((G / rowsum(E)) Wv^T + bv) * scale @ Wo^T + bo

Sharding: rows of H/T (i dimension) split across 8 cores, 1024 rows each;
H (both orientations) and the small dim x dim weights replicated.

Performance shape: the three N*R*D matmuls (distance, logits, E@H) plus the
rowsum run as fp8e4 DoubleRow matmuls (two 128-deep contraction planes per
instruction, PE double-pumped). Operand layout is [128, plane, X] so a
[:, 2u:2u+2, x] slice gives the [K,2,M] shape DoubleRow expects. PSUM banks
alternate between consecutive matmuls (same-bank successors serialize).
fp8 accuracy safeguards:
  - xx augmentation is centered at 512 and carried in two rows (value +
    residual) so e4m3's 3-bit mantissa does not perturb the distance scale;
  - Q2 is scaled by 16 into fp8's normal range; exp() applies scale 1/16
    and bias -3 so E stays well under the TRN fp8e4 max of 240;
  - the small dim x dim projection chains stay bf16.
"""

import math
import os
import sys

import numpy as np

for _p in ("/opt/trn_rl_repo", "/root/.axon_site", "/root/.axon_site/_ro/trn_rl_repo"):
    if os.path.isdir(_p) and _p not in sys.path:
        sys.path.append(_p)

import ml_dtypes

import concourse.bass as bass
import concourse.mybir as mybir
import concourse.tile as tile
from concourse import bacc, bass_utils

N = 8192          # total rows
D = 512           # feature dim
NCORES = 8
R = N // NCORES   # rows per core (1024)
P = 128           # partitions
KT = D // P       # 4 contraction planes
CH = 512          # free-dim chunk (one PSUM bank of f32)
NJC = N // CH     # 16 j-chunks
NJT = N // P      # 64 j-tiles
NPAIR = NJT // 2  # 32 j-tile pairs (DoubleRow granularity)
NIC = R // CH     # 2 i-chunks
NIT = R // P      # 8 i-tiles
JG = 2            # j-chunks per distance group (rotating PSUM banks)
NG = NJC // JG    # 8 distance groups
BF = mybir.dt.bfloat16
F32 = mybir.dt.float32
F8 = mybir.dt.float8e4
DR = mybir.MatmulPerfMode.DoubleRow
AF = mybir.ActivationFunctionType
ALU = mybir.AluOpType
Q2SCALE = 16.0 / math.sqrt(D)   # Q2 stored as 16*Q2_true in fp8
EXPSCALE = 1.0 / 16.0
EXPBIAS = -4.75

bf16 = ml_dtypes.bfloat16
f8e4 = ml_dtypes.float8_e4m3


def _emit(tc, io):
    nc = tc.nc
    from contextlib import ExitStack

    with ExitStack() as ctx:
        const = ctx.enter_context(tc.tile_pool(name="const", bufs=1))
        psum = ctx.enter_context(tc.tile_pool(name="psum", bufs=1, space="PSUM"))
        # attention-phase pools created up front so their SBUF space is
        # carved out early: their first DMAs must not wait on the early
        # pool's release.
        e_pool = ctx.enter_context(tc.tile_pool(name="ep", bufs=4))
        h_pool = ctx.enter_context(tc.tile_pool(name="hp", bufs=4))
        o_pool = ctx.enter_context(tc.tile_pool(name="op", bufs=2))

        # ---- small shared constants ----------------------------------------
        ones_f1 = const.tile([1, P], F32, name="onesf1")
        nc.vector.memset(ones_f1, 1.0)
        ones_b1 = const.tile([1, P], BF, name="onesb1")
        nc.vector.memset(ones_b1, 1.0)
        sixteen_col = const.tile([P, 1], F32, name="sixteencol")
        nc.vector.memset(sixteen_col, 16.0)
        expb_col = const.tile([P, 1], F32, name="expbcol")
        nc.vector.memset(expb_col, EXPBIAS)
        # [128,2,128] fp8 stationary whose first column is ones in both
        # planes: DoubleRow rowsum over a pair of 128-row tiles.
        onesw2 = const.tile([P, 2, P], F8, name="onesw2")
        nc.vector.memset(onesw2, 0.0)
        nc.vector.memset(onesw2[:, :, 0:1], 1.0)
        cst_1r = const.tile([1, R], F8, name="cst1r")
        nc.vector.memset(cst_1r, 1.0)
        cst_m4 = const.tile([1, R], F8, name="cstm4")
        nc.vector.memset(cst_m4, -4.0)
        cst_128 = const.tile([1, JG * CH], F8, name="cst128")
        nc.vector.memset(cst_128, 128.0)

        # ---- long-lived tensors (written early, read late) -----------------
        q2f8 = const.tile([P, KT, R], F8, name="q2f8")
        bv_row = const.tile([1, D], BF, name="bvrow")
        nc.sync.dma_start(bv_row, io["bvb"][:, :])
        bo_row = const.tile([1, D], BF, name="borow")
        nc.sync.dma_start(bo_row, io["bob"][:, :])
        WvT, WoT = [], []
        for m in range(KT):
            wvt_t = const.tile([P, D], BF, name=f"wvt{m}")
            nc.sync.dma_start(wvt_t, io["WvTb"][m * P:(m + 1) * P, :])
            WvT.append(wvt_t)
            wot_t = const.tile([P, D], BF, name=f"wot{m}")
            nc.sync.dma_start(wot_t, io["WoTb"][m * P:(m + 1) * P, :])
            WoT.append(wot_t)
        # resident fp8 H^T for the logits matmuls, loaded during the distance
        # phase (HBM is nearly idle there); quarter-row chunks keep single
        # HWDGE transfers small so the tt stream is not head-of-line blocked
        ht8 = const.tile([P, KT, N], F8, name="ht8")
        for k in range(KT):
            for q in range(4):
                nc.sync.dma_start(
                    ht8[:, k:k + 1, q * (N // 4):(q + 1) * (N // 4)],
                    io["HTf8"][k * P:(k + 1) * P,
                               q * (N // 4):(q + 1) * (N // 4)])
        GT = [const.tile([P, R], BF, name=f"gt{d_}") for d_ in range(KT)]
        YT = [const.tile([P, R], BF, name=f"yt{m}") for m in range(KT)]
        SNB = const.tile([P, R], F32, name="snb")
        scl_row = const.tile([1, R], F32, name="sclrow")
        scl_b = const.tile([1, R], BF, name="sclb")
        rs_row = const.tile([1, R], F32, name="rsrow")
        sn_row = const.tile([1, R], F32, name="snrow")

        # ---- early phases (scoped SBUF) ------------------------------------
        with tc.tile_pool(name="early", bufs=1) as early:
            tct = early.tile([P, KT, R], F8, name="tct")
            for k in range(KT):
                nc.sync.dma_start(tct[:, k:k + 1, :],
                                  io["TcTf8"][k * P:(k + 1) * P, :])
            # aug operand, 128-deep zero-padded plane0 + zero plane1 so the
            # aug matmul is a normal full-array DoubleRow instruction.
            # plane0 rows: r0=q_i, r1=res_i (pair with moving ones),
            # r2=1, r3=1 (pair with moving q_j, res_j), r4=-4 (pairs with
            # moving 128 -> -512 constant); q+res = -(xx-512)/2.
            aug_lhs = early.tile([P, 2, R], F8, name="auglhs")
            nc.vector.memset(aug_lhs, 0.0)
            nc.sync.dma_start(aug_lhs[2:3, 0:1, :], cst_1r)
            nc.sync.dma_start(aug_lhs[3:4, 0:1, :], cst_1r)
            nc.sync.dma_start(aug_lhs[4:5, 0:1, :], cst_m4)
            dsum = [early.tile([P, NJC], F32, name=f"dsum{it}")
                    for it in range(NIT)]

            with tc.tile_pool(name="sqp", bufs=3) as sq_pool, \
                 tc.tile_pool(name="ttp", bufs=2) as tt_pool, \
                 tc.tile_pool(name="dsp", bufs=3) as dist_pool, \
                 tc.tile_pool(name="augp", bufs=2) as aug_pool:

                # -- xx over this core's own rows -> aug_lhs rows 0/1 --------
                pssc = [psum.tile([P, CH], F32, tag="mm", bufs=3, name="psxxc")
                        for _ in range(NIC)]
                sqcs = [[None] * 2 for _ in range(NIC)]
                for ic in range(NIC):
                    for u in range(2):
                        sqc = sq_pool.tile([P, 2, CH], F8, tag=f"sq{ic}{u}",
                                           name="sqc")
                        for pl in range(2):
                            k = 2 * u + pl
                            eng = nc.vector if pl == 0 else nc.gpsimd
                            eng.tensor_mul(
                                sqc[:, pl:pl + 1, :],
                                tct[:, k:k + 1, ic * CH:(ic + 1) * CH],
                                tct[:, k:k + 1, ic * CH:(ic + 1) * CH])
                        sqcs[ic][u] = sqc
                for u in range(2):
                    for ic in range(NIC):
                        nc.tensor.matmul(pssc[ic], onesw2, sqcs[ic][u],
                                         start=(u == 0), stop=(u == 1),
                                         perf_mode=DR)
                for ic in range(NIC):
                    csl = slice(ic * CH, (ic + 1) * CH)
                    tv = sq_pool.tile([1, CH], F32, tag="tv", bufs=2,
                                      name="tvc")
                    nc.vector.tensor_scalar(tv, pssc[ic][0:1, :], -0.5, 256.0,
                                            op0=ALU.mult, op1=ALU.add)
                    xq = sq_pool.tile([1, CH], F8, tag="xqc", bufs=2,
                                      name="xqc")
                    nc.vector.tensor_copy(xq, tv)
                    xr = sq_pool.tile([1, CH], F8, tag="xrc", bufs=2,
                                      name="xrc")
                    nc.vector.tensor_sub(xr, tv, xq)
                    nc.sync.dma_start(aug_lhs[0:1, 0:1, csl], xq)
                    nc.sync.dma_start(aug_lhs[1:2, 0:1, csl], xr)

                def load_group(jg):
                    tts = []
                    for jj in range(JG):
                        jc = jg * JG + jj
                        tt_t = tt_pool.tile([P, KT, CH], F8, tag=f"tt{jj}",
                                            name="ttd")
                        for k in range(KT):
                            nc.sync.dma_start(
                                tt_t[:, k:k + 1, :],
                                io["TTf8"][k * P:(k + 1) * P,
                                           jc * CH:(jc + 1) * CH])
                        tts.append(tt_t)
                    return tts

                def xx_chain(jg, tts):
                    # squares on DVE+GPSIMD; xx row via DoubleRow ones-matmul;
                    # value+residual rows land in augg plane0 via SBUF DMA.
                    augg = aug_pool.tile([P, 2, JG * CH], F8, tag="augg",
                                         name="augg")
                    nc.vector.memset(augg, 0.0)
                    nc.vector.memset(augg[0:1, 0:1, :], 1.0)
                    nc.sync.dma_start(augg[1:2, 0:1, :], cst_1r)
                    nc.sync.dma_start(augg[4:5, 0:1, :], cst_128)
                    pxx = [psum.tile([P, CH], F32, tag="mm", bufs=3,
                                     name="psxx") for _ in range(JG)]
                    sqs = [[None] * 2 for _ in range(JG)]
                    for jj in range(JG):
                        for u in range(2):
                            sq = sq_pool.tile([P, 2, CH], F8, tag=f"sq{jj}{u}",
                                              name="sq")
                            for pl in range(2):
                                k = 2 * u + pl
                                eng = nc.vector if pl == 0 else nc.gpsimd
                                eng.tensor_mul(sq[:, pl:pl + 1, :],
                                               tts[jj][:, k:k + 1, :],
                                               tts[jj][:, k:k + 1, :])
                            sqs[jj][u] = sq
                    for u in range(2):
                        for jj in range(JG):
                            nc.tensor.matmul(pxx[jj], onesw2, sqs[jj][u],
                                             start=(u == 0), stop=(u == 1),
                                             perf_mode=DR)
                    for jj in range(JG):
                        tv = sq_pool.tile([1, CH], F32, tag="tvj", bufs=2,
                                          name="tvj")
                        nc.vector.tensor_scalar(tv, pxx[jj][0:1, :], -0.5,
                                                256.0, op0=ALU.mult,
                                                op1=ALU.add)
                        xq = sq_pool.tile([1, CH], F8, tag="xq", bufs=2,
                                          name="xq")
                        nc.vector.tensor_copy(xq, tv)
                        xr = sq_pool.tile([1, CH], F8, tag="xr", bufs=2,
                                          name="xr")
                        nc.vector.tensor_sub(xr, tv, xq)
                        nc.sync.dma_start(
                            augg[2:3, 0:1, jj * CH:(jj + 1) * CH], xq)
                        nc.sync.dma_start(
                            augg[3:4, 0:1, jj * CH:(jj + 1) * CH], xr)
                    return augg

                def d2_group(jg, tts, augg):
                    for it in range(NIT):
                        # alternate bank pairs per it so the next iteration
                        # never waits on this one's drains
                        base = 2 * (it % 2)
                        pd = [psum.tile([P, CH], F32, tag=f"g{base + jj}",
                                        name=f"psd{jj}") for jj in range(JG)]
                        for u in range(2):
                            for jj in range(JG):
                                nc.tensor.matmul(
                                    pd[jj],
                                    tct[:, 2 * u:2 * u + 2,
                                        it * P:(it + 1) * P],
                                    tts[jj][:, 2 * u:2 * u + 2, :],
                                    start=(u == 0), stop=False, perf_mode=DR)
                        for jj in range(JG):
                            nc.tensor.matmul(
                                pd[jj], aug_lhs[:, :, it * P:(it + 1) * P],
                                augg[:, :, jj * CH:(jj + 1) * CH],
                                start=False, stop=True, perf_mode=DR)
                        for jj in range(JG):
                            jc = jg * JG + jj
                            # sqrt(dist2 + 16) straight from PSUM: the +16
                            # keeps the (fp8-noisy) diagonal positive; the
                            # systematic +8/dist shift is corrected
                            # analytically in the scale computation below.
                            dist_t = dist_pool.tile([P, CH], BF, tag="dist",
                                                    name="distt")
                            nc.scalar.activation(
                                dist_t, pd[jj], AF.Sqrt, scale=-2.0,
                                bias=sixteen_col,
                                accum_out=dsum[it][:, jc:jc + 1])

                tts_cur = load_group(0)
                augg_cur = xx_chain(0, tts_cur)
                tts_next = load_group(1)
                augg_next = xx_chain(1, tts_next)
                d2_group(0, tts_cur, augg_cur)
                tts_cur, augg_cur = tts_next, augg_next

                # -- Q chain (independent; fills PE while group 2 loads) -----
                with tc.tile_pool(name="qpool", bufs=1) as qpool:
                    HcT, WqT, Wk = [], [], []
                    for k in range(KT):
                        hct_t = qpool.tile([P, R], BF, name=f"hct{k}")
                        nc.sync.dma_start(hct_t,
                                          io["HcTb"][k * P:(k + 1) * P, :])
                        HcT.append(hct_t)
                        wqt_t = qpool.tile([P, D], BF, name=f"wqt{k}")
                        nc.sync.dma_start(wqt_t,
                                          io["WqTb"][k * P:(k + 1) * P, :])
                        WqT.append(wqt_t)
                        wk_t = qpool.tile([P, D], BF, name=f"wk{k}")
                        nc.sync.dma_start(wk_t,
                                          io["Wkb"][k * P:(k + 1) * P, :])
                        Wk.append(wk_t)
                    bq_sb = []
                    for m in range(KT):
                        b_t = qpool.tile([P, 1], F32, name=f"bq{m}")
                        nc.sync.dma_start(b_t, io["bqf"][m * P:(m + 1) * P, :])
                        bq_sb.append(b_t)
                    QT = [qpool.tile([P, R], BF, name=f"qt{m}")
                          for m in range(KT)]
                    for m in range(KT):
                        pq = [psum.tile([P, CH], F32, tag="mm", bufs=3,
                                        name="psq") for _ in range(NIC)]
                        for d_ in range(KT):
                            for ic in range(NIC):
                                nc.tensor.matmul(
                                    pq[ic], WqT[d_][:, m * P:(m + 1) * P],
                                    HcT[d_][:, ic * CH:(ic + 1) * CH],
                                    start=(d_ == 0), stop=(d_ == KT - 1))
                        for ic in range(NIC):
                            nc.scalar.activation(
                                QT[m][:, ic * CH:(ic + 1) * CH], pq[ic],
                                AF.Identity, bias=bq_sb[m])
                    for k in range(KT):
                        pq2 = [psum.tile([P, CH], F32, tag="mm", bufs=3,
                                         name="psq2") for _ in range(NIC)]
                        for m in range(KT):
                            for ic in range(NIC):
                                nc.tensor.matmul(
                                    pq2[ic], Wk[m][:, k * P:(k + 1) * P],
                                    QT[m][:, ic * CH:(ic + 1) * CH],
                                    start=(m == 0), stop=(m == KT - 1))
                        for ic in range(NIC):
                            nc.scalar.activation(
                                q2f8[:, k:k + 1, ic * CH:(ic + 1) * CH],
                                pq2[ic], AF.Copy, scale=Q2SCALE)

                # -- distance groups, software pipelined ---------------------
                for jg in range(1, NG):
                    if jg + 1 < NG:
                        tts_next = load_group(jg + 1)
                        augg_next = xx_chain(jg + 1, tts_next)
                    else:
                        tts_next = augg_next = None
                    d2_group(jg, tts_cur, augg_cur)
                    tts_cur, augg_cur = tts_next, augg_next

            with tc.tile_pool(name="scl", bufs=1, space="DRAM") as dram:
                scl_dram = dram.tile([R, 1], F32, name="scldram")
                for it in range(NIT):
                    red = early.tile([P, 1], F32, name=f"red{it}")
                    nc.vector.reduce_sum(red, dsum[it],
                                         axis=mybir.AxisListType.X)
                    mcol = early.tile([P, 1], F32, name=f"mcol{it}")
                    nc.vector.tensor_scalar(mcol, red, 1.0 / N, None,
                                            op0=ALU.mult)
                    ucol = early.tile([P, 1], F32, name=f"ucol{it}")
                    nc.vector.reciprocal(ucol, mcol)
                    # measured mean of sqrt(dist2+16) = true mean + 8/m +
                    # diag excess 4/N; scale = 1/(1 + m - 8/m - 0.000488)
                    uc2 = early.tile([P, 1], F32, name=f"uc2{it}")
                    nc.vector.tensor_scalar(uc2, ucol, -8.0, 0.999512,
                                            op0=ALU.mult, op1=ALU.add)
                    tmp = early.tile([P, 1], F32, name=f"sctmp{it}")
                    nc.vector.tensor_add(tmp, mcol, uc2)
                    scol = early.tile([P, 1], F32, name=f"scol{it}")
                    nc.vector.reciprocal(scol, tmp)
                    nc.sync.dma_start(scl_dram[it * P:(it + 1) * P, :], scol)
                nc.sync.dma_start(
                    scl_row, scl_dram.rearrange("(a p) c -> a (p c)", a=1))
                nc.vector.tensor_copy(scl_b, scl_row)


        # ---- attention passes: pipelined logits(pair s) | G/rowsum(s-2) ----
        def attention_pass(ic):
            csl = slice(ic * CH, (ic + 1) * CH)
            g_ps = [psum.tile([P, CH], F32, tag=f"g{d_}", name=f"gps{d_}")
                    for d_ in range(KT)]
            rs_ps = psum.tile([P, CH], F32, tag="rowps", name="rsps")
            # two-deep pair pipeline: G/rowsum lag the logits by 2 pairs so
            # the exp of pair s-2 is long done when its G matmuls issue
            pipe = []  # [(e2_t, h2_t, s), ...]

            def g_mm(lag, k, stop=False):
                nc.tensor.matmul(g_ps[k], lag[1][:, :, k * P:(k + 1) * P],
                                 lag[0], start=(lag[2] == 0), stop=stop,
                                 perf_mode=DR)

            for s in range(NPAIR):
                h2_t = h_pool.tile([P, 2, D], F8, tag="h", name="h2t")
                nc.sync.dma_start(h2_t[:, 0:1, :],
                                  io["Hf8"][(2 * s) * P:(2 * s + 1) * P, :])
                nc.sync.dma_start(h2_t[:, 1:2, :],
                                  io["Hf8"][(2 * s + 1) * P:(2 * s + 2) * P, :])
                e2_t = e_pool.tile([P, 2, CH], F8, tag="e", name="e2t")
                st_a = psum.tile([P, CH], F32, tag="mm", bufs=3, name="sta")
                st_b = psum.tile([P, CH], F32, tag="mm", bufs=3, name="stb")
                lag = pipe[0] if len(pipe) == 2 else None
                nc.tensor.matmul(st_a, ht8[:, 0:2, (2 * s) * P:(2 * s + 1) * P],
                                 q2f8[:, 0:2, csl], start=True, stop=False,
                                 perf_mode=DR)
                if lag is not None:
                    g_mm(lag, 0)
                nc.tensor.matmul(st_a, ht8[:, 2:4, (2 * s) * P:(2 * s + 1) * P],
                                 q2f8[:, 2:4, csl], start=False, stop=True,
                                 perf_mode=DR)
                if lag is not None:
                    g_mm(lag, 1)
                nc.scalar.activation(e2_t[:, 0:1, :], st_a, AF.Exp,
                                     scale=EXPSCALE, bias=expb_col)
                nc.tensor.matmul(st_b,
                                 ht8[:, 0:2, (2 * s + 1) * P:(2 * s + 2) * P],
                                 q2f8[:, 0:2, csl], start=True, stop=False,
                                 perf_mode=DR)
                if lag is not None:
                    g_mm(lag, 2)
                nc.tensor.matmul(st_b,
                                 ht8[:, 2:4, (2 * s + 1) * P:(2 * s + 2) * P],
                                 q2f8[:, 2:4, csl], start=False, stop=True,
                                 perf_mode=DR)
                if lag is not None:
                    g_mm(lag, 3)
                    nc.tensor.matmul(rs_ps, onesw2, lag[0],
                                     start=(lag[2] == 0), stop=False,
                                     perf_mode=DR)
                    pipe.pop(0)
                nc.scalar.activation(e2_t[:, 1:2, :], st_b, AF.Exp,
                                     scale=EXPSCALE, bias=expb_col)
                pipe.append((e2_t, h2_t, s))
            for (e2_t, h2_t, s) in pipe:
                last = s == NPAIR - 1
                for k in range(KT):
                    nc.tensor.matmul(g_ps[k], h2_t[:, :, k * P:(k + 1) * P],
                                     e2_t, start=(s == 0), stop=last,
                                     perf_mode=DR)
                nc.tensor.matmul(rs_ps, onesw2, e2_t, start=(s == 0),
                                 stop=last, perf_mode=DR)
            # drain accumulators promptly so the next pass can claim the banks
            for d_ in range(KT):
                nc.scalar.activation(GT[d_][:, csl], g_ps[d_], AF.Copy)
            nc.vector.tensor_copy(rs_row[0:1, csl], rs_ps[0:1, :])

        def tail(ic):
            csl = slice(ic * CH, (ic + 1) * CH)
            nc.vector.reciprocal(sn_row[0:1, csl], rs_row[0:1, csl])
            nc.vector.tensor_mul(sn_row[0:1, csl], sn_row[0:1, csl],
                                 scl_row[0:1, csl])
            ps_snb = psum.tile([P, CH], F32, tag="mm", bufs=3, name="pssnb")
            nc.tensor.matmul(ps_snb, ones_f1, sn_row[0:1, csl],
                             start=True, stop=True)
            nc.vector.tensor_copy(SNB[:, csl], ps_snb)
            for d_ in range(KT):
                nc.vector.tensor_mul(GT[d_][:, csl], GT[d_][:, csl],
                                     SNB[:, csl])
            # Y^T = Wv Gn^T + (bv x scale): two m-chains in flight
            for m0 in range(0, KT, 2):
                py = [psum.tile([P, CH], F32, tag="mm", bufs=3, name="psy")
                      for _ in range(2)]
                for d_ in range(KT):
                    for u in range(2):
                        m = m0 + u
                        nc.tensor.matmul(py[u], WvT[d_][:, m * P:(m + 1) * P],
                                         GT[d_][:, csl],
                                         start=(d_ == 0), stop=False)
                for u in range(2):
                    m = m0 + u
                    nc.tensor.matmul(py[u], bv_row[0:1, m * P:(m + 1) * P],
                                     scl_b[0:1, csl], start=False, stop=True)
                for u in range(2):
                    m = m0 + u
                    nc.scalar.activation(YT[m][:, csl], py[u], AF.Copy)
            # out = Y Wo^T + bo for this chunk's 4 i-tiles, chains in pairs
            for it0 in range(ic * 4, (ic + 1) * 4, 2):
                po = [psum.tile([P, CH], F32, tag="mm", bufs=3, name="pso")
                      for _ in range(2)]
                for m in range(KT):
                    for u in range(2):
                        it = it0 + u
                        nc.tensor.matmul(po[u], YT[m][:, it * P:(it + 1) * P],
                                         WoT[m], start=(m == 0), stop=False)
                for u in range(2):
                    nc.tensor.matmul(po[u], ones_b1, bo_row,
                                     start=False, stop=True)
                for u in range(2):
                    it = it0 + u
                    o_t = o_pool.tile([P, D], F32, tag="o", name="ot")
                    nc.scalar.activation(o_t, po[u], AF.Copy)
                    nc.sync.dma_start(io["OUT"][it * P:(it + 1) * P, :], o_t)

        attention_pass(0)
        attention_pass(1)
        tail(0)
        tail(1)


_NC_CACHE = None


def _build():
    global _NC_CACHE
    if _NC_CACHE is not None:
        return _NC_CACHE
    nc = bacc.Bacc("TRN2", target_bir_lowering=False, debug=False,
                   enable_asserts=False, num_devices=NCORES)
    io = {
        "HTf8": nc.dram_tensor("HTf8", [D, N], F8, kind="ExternalInput").ap(),
        "Hf8": nc.dram_tensor("Hf8", [N, D], F8, kind="ExternalInput").ap(),
        "TTf8": nc.dram_tensor("TTf8", [D, N], F8, kind="ExternalInput").ap(),
        "TcTf8": nc.dram_tensor("TcTf8", [D, R], F8,
                                kind="ExternalInput").ap(),
        "HcTb": nc.dram_tensor("HcTb", [D, R], BF, kind="ExternalInput").ap(),
        "WqTb": nc.dram_tensor("WqTb", [D, D], BF, kind="ExternalInput").ap(),
        "Wkb": nc.dram_tensor("Wkb", [D, D], BF, kind="ExternalInput").ap(),
        "WvTb": nc.dram_tensor("WvTb", [D, D], BF, kind="ExternalInput").ap(),
        "WoTb": nc.dram_tensor("WoTb", [D, D], BF, kind="ExternalInput").ap(),
        "bqf": nc.dram_tensor("bqf", [D, 1], F32, kind="ExternalInput").ap(),
        "bvb": nc.dram_tensor("bvb", [1, D], BF, kind="ExternalInput").ap(),
        "bob": nc.dram_tensor("bob", [1, D], BF, kind="ExternalInput").ap(),
        "OUT": nc.dram_tensor("OUT", [R, D], F32, kind="ExternalOutput").ap(),
    }
    with tile.TileContext(nc) as tc:
        _emit(tc, io)
    nc.compile()
    _NC_CACHE = nc
    return nc


LAST_RESULTS = None


def _to_f8(a):
    return np.clip(a, -240.0, 240.0).astype(f8e4)


def kernel(H, T, Wq, bq, Wk, bk, Wv, bv, Wo, bo):
    global LAST_RESULTS
    H = np.ascontiguousarray(np.asarray(H, np.float32))
    T = np.ascontiguousarray(np.asarray(T, np.float32))

    HT = np.ascontiguousarray(H.T)
    TT = np.ascontiguousarray(T.T)
    HTb = HT.astype(bf16)
    shared = {
        "HTf8": _to_f8(HT),
        "Hf8": _to_f8(H),
        "TTf8": _to_f8(TT),
        "WqTb": np.ascontiguousarray(np.asarray(Wq, np.float32).T).astype(bf16),
        "Wkb": np.ascontiguousarray(np.asarray(Wk, np.float32)).astype(bf16),
        "WvTb": np.ascontiguousarray(np.asarray(Wv, np.float32).T).astype(bf16),
        "WoTb": np.ascontiguousarray(np.asarray(Wo, np.float32).T).astype(bf16),
        "bqf": np.asarray(bq, np.float32).reshape(D, 1).copy(),
        "bvb": np.asarray(bv, np.float32).reshape(1, D).astype(bf16),
        "bob": np.asarray(bo, np.float32).reshape(1, D).astype(bf16),
    }
    in_maps = []
    for c in range(NCORES):
        m = dict(shared)
        m["TcTf8"] = np.ascontiguousarray(shared["TTf8"][:, c * R:(c + 1) * R])
        m["HcTb"] = np.ascontiguousarray(HTb[:, c * R:(c + 1) * R])
        in_maps.append(m)

    nc = _build()
    res = bass_utils.run_bass_kernel_spmd(nc, in_maps,
                                          core_ids=list(range(NCORES)))
    LAST_RESULTS = res
    out = np.concatenate([res.results[c]["OUT"] for c in range(NCORES)],
                         axis=0)
    return np.ascontiguousarray(out.astype(np.float32))


# revision 11
# speedup vs baseline: 1.5400x; 1.0341x over previous
"""Trainium2 Bass kernel for a causal-attention-like module (fp8 DoubleRow).

Math (reassociated from the reference nn.Module):
    dist[i,j] = sqrt(max(|T_i|^2 + |T_j|^2 - 2 T_i.T_j, 0) + 1e-8)
    scale_i   = 1 / (1 + mean_j dist[i,j])
    Q2        = (H Wq^T + bq) Wk / sqrt(d)         # bk cancels inside softmax
    E[i,j]    = exp(Q2[i,:] . H[j,:] - 3)          # -3 shift cancels in rownorm
    G         = E @ H                              # unnormalized
    out       = ((G / rowsum(E)) Wv^T + bv) * scale @ Wo^T + bo

Sharding: rows of H/T (i dimension) split across 8 cores, 1024 rows each;
H (both orientations) and the small dim x dim weights replicated.

Performance shape: the three N*R*D matmuls (distance, logits, E@H) plus the
rowsum run as fp8e4 DoubleRow matmuls (two 128-deep contraction planes per
instruction, PE double-pumped). Operand layout is [128, plane, X] so a
[:, 2u:2u+2, x] slice gives the [K,2,M] shape DoubleRow expects. PSUM banks
alternate between consecutive matmuls (same-bank successors serialize).
fp8 accuracy safeguards:
  - xx augmentation is centered at 512 and carried in two rows (value +
    residual) so e4m3's 3-bit mantissa does not perturb the distance scale;
  - Q2 is scaled by 16 into fp8's normal range; exp() applies scale 1/16
    and bias -3 so E stays well under the TRN fp8e4 max of 240;
  - the small dim x dim projection chains stay bf16.
"""

import math
import os
import sys

import numpy as np

for _p in ("/opt/trn_rl_repo", "/root/.axon_site", "/root/.axon_site/_ro/trn_rl_repo"):
    if os.path.isdir(_p) and _p not in sys.path:
        sys.path.append(_p)

import ml_dtypes

import concourse.bass as bass
import concourse.mybir as mybir
import concourse.tile as tile
from concourse import bacc, bass_utils

N = 8192          # total rows
D = 512           # feature dim
NCORES = 8
R = N // NCORES   # rows per core (1024)
P = 128           # partitions
KT = D // P       # 4 contraction planes
CH = 512          # free-dim chunk (one PSUM bank of f32)
NJC = N // CH     # 16 j-chunks
NJT = N // P      # 64 j-tiles
NPAIR = NJT // 2  # 32 j-tile pairs (DoubleRow granularity)
NIC = R // CH     # 2 i-chunks
NIT = R // P      # 8 i-tiles
JG = 2            # j-chunks per distance group (rotating PSUM banks)
NG = NJC // JG    # 8 distance groups
BF = mybir.dt.bfloat16
F32 = mybir.dt.float32
F8 = mybir.dt.float8e4
DR = mybir.MatmulPerfMode.DoubleRow
AF = mybir.ActivationFunctionType
ALU = mybir.AluOpType
Q2SCALE = 16.0 / math.sqrt(D)   # Q2 stored as 16*Q2_true in fp8
EXPSCALE = 1.0 / 16.0
EXPBIAS = -4.75

bf16 = ml_dtypes.bfloat16
f8e4 = ml_dtypes.float8_e4m3


def _emit(tc, io):
    nc = tc.nc
    from contextlib import ExitStack

    with ExitStack() as ctx:
        const = ctx.enter_context(tc.tile_pool(name="const", bufs=1))
        psum = ctx.enter_context(tc.tile_pool(name="psum", bufs=1, space="PSUM"))
        # attention-phase pools created up front so their SBUF space is
        # carved out early: their first DMAs must not wait on the early
        # pool's release.
        e_pool = ctx.enter_context(tc.tile_pool(name="ep", bufs=4))
        h_pool = ctx.enter_context(tc.tile_pool(name="hp", bufs=4))
        o_pool = ctx.enter_context(tc.tile_pool(name="op", bufs=2))

        # ---- small shared constants ----------------------------------------
        ones_f1 = const.tile([1, P], F32, name="onesf1")
        nc.vector.memset(ones_f1, 1.0)
        ones_b1 = const.tile([1, P], BF, name="onesb1")
        nc.vector.memset(ones_b1, 1.0)
        sixteen_col = const.tile([P, 1], F32, name="sixteencol")
        nc.vector.memset(sixteen_col, 16.0)
        expb_col = const.tile([P, 1], F32, name="expbcol")
        nc.vector.memset(expb_col, EXPBIAS)
        # [128,2,128] fp8 stationary whose first column is ones in both
        # planes: DoubleRow rowsum over a pair of 128-row tiles.
        onesw2 = const.tile([P, 2, P], F8, name="onesw2")
        nc.vector.memset(onesw2, 0.0)
        nc.vector.memset(onesw2[:, :, 0:1], 1.0)
        cst_1r = const.tile([1, R], F8, name="cst1r")
        nc.vector.memset(cst_1r, 1.0)
        cst_m4 = const.tile([1, R], F8, name="cstm4")
        nc.vector.memset(cst_m4, -4.0)
        cst_128 = const.tile([1, JG * CH], F8, name="cst128")
        nc.vector.memset(cst_128, 128.0)

        # ---- long-lived tensors (written early, read late) -----------------
        q2f8 = const.tile([P, KT, R], F8, name="q2f8")
        bv_row = const.tile([1, D], BF, name="bvrow")
        bo_row = const.tile([1, D], BF, name="borow")
        WvT, WoT = [], []
        for m in range(KT):
            wvt_t = const.tile([P, D], BF, name=f"wvt{m}")
            WvT.append(wvt_t)
            wot_t = const.tile([P, D], BF, name=f"wot{m}")
            WoT.append(wot_t)
        # resident fp8 H^T for the logits matmuls; allocated here but DMA'd
        # in pieces during the distance phase so the dist-critical tct/tt
        # loads are never queued behind this 4MB stream.
        ht8 = const.tile([P, KT, N], F8, name="ht8")
        ht8_chunks = [(k, q) for k in range(KT) for q in range(4)]

        def emit_ht8(n):
            for _ in range(n):
                if not ht8_chunks:
                    return
                k, q = ht8_chunks.pop(0)
                nc.sync.dma_start(
                    ht8[:, k:k + 1, q * (N // 4):(q + 1) * (N // 4)],
                    io["HTf8"][k * P:(k + 1) * P,
                               q * (N // 4):(q + 1) * (N // 4)])

        def emit_tailw():
            nc.sync.dma_start(bv_row, io["bvb"][:, :])
            nc.sync.dma_start(bo_row, io["bob"][:, :])
            for m in range(KT):
                nc.sync.dma_start(WvT[m], io["WvTb"][m * P:(m + 1) * P, :])
                nc.sync.dma_start(WoT[m], io["WoTb"][m * P:(m + 1) * P, :])
        GT = [const.tile([P, R], BF, name=f"gt{d_}") for d_ in range(KT)]
        YT = [const.tile([P, R], BF, name=f"yt{m}") for m in range(KT)]
        SNB = const.tile([P, R], F32, name="snb")
        scl_row = const.tile([1, R], F32, name="sclrow")
        scl_b = const.tile([1, R], BF, name="sclb")
        rs_row = const.tile([1, R], F32, name="rsrow")
        sn_row = const.tile([1, R], F32, name="snrow")

        # ---- early phases (scoped SBUF) ------------------------------------
        with tc.tile_pool(name="early", bufs=1) as early:
            tct = early.tile([P, KT, R], F8, name="tct")
            for k in range(KT):
                nc.sync.dma_start(tct[:, k:k + 1, :],
                                  io["TcTf8"][k * P:(k + 1) * P, :])
            # aug operand, 128-deep zero-padded plane0 + zero plane1 so the
            # aug matmul is a normal full-array DoubleRow instruction.
            # plane0 rows: r0=q_i, r1=res_i (pair with moving ones),
            # r2=1, r3=1 (pair with moving q_j, res_j), r4=-4 (pairs with
            # moving 128 -> -512 constant); q+res = -(xx-512)/2.
            aug_lhs = early.tile([P, 2, R], F8, name="auglhs")
            nc.vector.memset(aug_lhs, 0.0)
            nc.sync.dma_start(aug_lhs[2:3, 0:1, :], cst_1r)
            nc.sync.dma_start(aug_lhs[3:4, 0:1, :], cst_1r)
            nc.sync.dma_start(aug_lhs[4:5, 0:1, :], cst_m4)
            dsum = [early.tile([P, NJC], F32, name=f"dsum{it}")
                    for it in range(NIT)]

            with tc.tile_pool(name="sqp", bufs=3) as sq_pool, \
                 tc.tile_pool(name="ttp", bufs=2) as tt_pool, \
                 tc.tile_pool(name="dsp", bufs=3) as dist_pool, \
                 tc.tile_pool(name="augp", bufs=2) as aug_pool:

                # -- xx over this core's own rows -> aug_lhs rows 0/1 --------
                pssc = [psum.tile([P, CH], F32, tag="mm", bufs=3, name="psxxc")
                        for _ in range(NIC)]
                sqcs = [[None] * 2 for _ in range(NIC)]
                for ic in range(NIC):
                    for u in range(2):
                        sqc = sq_pool.tile([P, 2, CH], F8, tag=f"sq{ic}{u}",
                                           name="sqc")
                        for pl in range(2):
                            k = 2 * u + pl
                            eng = nc.vector if pl == 0 else nc.gpsimd
                            eng.tensor_mul(
                                sqc[:, pl:pl + 1, :],
                                tct[:, k:k + 1, ic * CH:(ic + 1) * CH],
                                tct[:, k:k + 1, ic * CH:(ic + 1) * CH])
                        sqcs[ic][u] = sqc
                for u in range(2):
                    for ic in range(NIC):
                        nc.tensor.matmul(pssc[ic], onesw2, sqcs[ic][u],
                                         start=(u == 0), stop=(u == 1),
                                         perf_mode=DR)
                for ic in range(NIC):
                    csl = slice(ic * CH, (ic + 1) * CH)
                    tv = sq_pool.tile([1, CH], F32, tag="tv", bufs=2,
                                      name="tvc")
                    nc.vector.tensor_scalar(tv, pssc[ic][0:1, :], -0.5, 256.0,
                                            op0=ALU.mult, op1=ALU.add)
                    xq = sq_pool.tile([1, CH], F8, tag="xqc", bufs=2,
                                      name="xqc")
                    nc.vector.tensor_copy(xq, tv)
                    xr = sq_pool.tile([1, CH], F8, tag="xrc", bufs=2,
                                      name="xrc")
                    nc.vector.tensor_sub(xr, tv, xq)
                    nc.sync.dma_start(aug_lhs[0:1, 0:1, csl], xq)
                    nc.sync.dma_start(aug_lhs[1:2, 0:1, csl], xr)

                def load_group(jg):
                    tts = []
                    for jj in range(JG):
                        jc = jg * JG + jj
                        tt_t = tt_pool.tile([P, KT, CH], F8, tag=f"tt{jj}",
                                            name="ttd")
                        for k in range(KT):
                            nc.sync.dma_start(
                                tt_t[:, k:k + 1, :],
                                io["TTf8"][k * P:(k + 1) * P,
                                           jc * CH:(jc + 1) * CH])
                        tts.append(tt_t)
                    return tts

                def xx_chain(jg, tts):
                    # squares on DVE+GPSIMD; xx row via DoubleRow ones-matmul;
                    # value+residual rows land in augg plane0 via SBUF DMA.
                    augg = aug_pool.tile([P, 2, JG * CH], F8, tag="augg",
                                         name="augg")
                    nc.vector.memset(augg, 0.0)
                    nc.vector.memset(augg[0:1, 0:1, :], 1.0)
                    nc.sync.dma_start(augg[1:2, 0:1, :], cst_1r)
                    nc.sync.dma_start(augg[4:5, 0:1, :], cst_128)
                    pxx = [psum.tile([P, CH], F32, tag="mm", bufs=3,
                                     name="psxx") for _ in range(JG)]
                    sqs = [[None] * 2 for _ in range(JG)]
                    for jj in range(JG):
                        for u in range(2):
                            sq = sq_pool.tile([P, 2, CH], F8, tag=f"sq{jj}{u}",
                                              name="sq")
                            for pl in range(2):
                                k = 2 * u + pl
                                eng = nc.vector if pl == 0 else nc.gpsimd
                                eng.tensor_mul(sq[:, pl:pl + 1, :],
                                               tts[jj][:, k:k + 1, :],
                                               tts[jj][:, k:k + 1, :])
                            sqs[jj][u] = sq
                    for u in range(2):
                        for jj in range(JG):
                            nc.tensor.matmul(pxx[jj], onesw2, sqs[jj][u],
                                             start=(u == 0), stop=(u == 1),
                                             perf_mode=DR)
                    for jj in range(JG):
                        tv = sq_pool.tile([1, CH], F32, tag="tvj", bufs=2,
                                          name="tvj")
                        nc.vector.tensor_scalar(tv, pxx[jj][0:1, :], -0.5,
                                                256.0, op0=ALU.mult,
                                                op1=ALU.add)
                        xq = sq_pool.tile([1, CH], F8, tag="xq", bufs=2,
                                          name="xq")
                        nc.vector.tensor_copy(xq, tv)
                        xr = sq_pool.tile([1, CH], F8, tag="xr", bufs=2,
                                          name="xr")
                        nc.vector.tensor_sub(xr, tv, xq)
                        nc.sync.dma_start(
                            augg[2:3, 0:1, jj * CH:(jj + 1) * CH], xq)
                        nc.sync.dma_start(
                            augg[3:4, 0:1, jj * CH:(jj + 1) * CH], xr)
                    return augg

                def d2_group(jg, tts, augg):
                    for it in range(NIT):
                        # alternate bank pairs per it so the next iteration
                        # never waits on this one's drains
                        base = 2 * (it % 2)
                        pd = [psum.tile([P, CH], F32, tag=f"g{base + jj}",
                                        name=f"psd{jj}") for jj in range(JG)]
                        for u in range(2):
                            for jj in range(JG):
                                nc.tensor.matmul(
                                    pd[jj],
                                    tct[:, 2 * u:2 * u + 2,
                                        it * P:(it + 1) * P],
                                    tts[jj][:, 2 * u:2 * u + 2, :],
                                    start=(u == 0), stop=False, perf_mode=DR)
                        for jj in range(JG):
                            nc.tensor.matmul(
                                pd[jj], aug_lhs[:, :, it * P:(it + 1) * P],
                                augg[:, :, jj * CH:(jj + 1) * CH],
                                start=False, stop=True, perf_mode=DR)
                        for jj in range(JG):
                            jc = jg * JG + jj
                            # sqrt(dist2 + 16) straight from PSUM: the +16
                            # keeps the (fp8-noisy) diagonal positive; the
                            # systematic +8/dist shift is corrected
                            # analytically in the scale computation below.
                            dist_t = dist_pool.tile([P, CH], BF, tag="dist",
                                                    name="distt")
                            nc.scalar.activation(
                                dist_t, pd[jj], AF.Sqrt, scale=-2.0,
                                bias=sixteen_col,
                                accum_out=dsum[it][:, jc:jc + 1])

                tts_cur = load_group(0)
                augg_cur = xx_chain(0, tts_cur)
                tts_next = load_group(1)
                augg_next = xx_chain(1, tts_next)
                emit_ht8(2)
                d2_group(0, tts_cur, augg_cur)
                tts_cur, augg_cur = tts_next, augg_next

                # -- Q chain (independent; fills PE while group 2 loads) -----
                with tc.tile_pool(name="qpool", bufs=1) as qpool:
                    HcT, WqT, Wk = [], [], []
                    for k in range(KT):
                        hct_t = qpool.tile([P, R], BF, name=f"hct{k}")
                        nc.sync.dma_start(hct_t,
                                          io["HcTb"][k * P:(k + 1) * P, :])
                        HcT.append(hct_t)
                        wqt_t = qpool.tile([P, D], BF, name=f"wqt{k}")
                        nc.sync.dma_start(wqt_t,
                                          io["WqTb"][k * P:(k + 1) * P, :])
                        WqT.append(wqt_t)
                        wk_t = qpool.tile([P, D], BF, name=f"wk{k}")
                        nc.sync.dma_start(wk_t,
                                          io["Wkb"][k * P:(k + 1) * P, :])
                        Wk.append(wk_t)
                    bq_sb = []
                    for m in range(KT):
                        b_t = qpool.tile([P, 1], F32, name=f"bq{m}")
                        nc.sync.dma_start(b_t, io["bqf"][m * P:(m + 1) * P, :])
                        bq_sb.append(b_t)
                    QT = [qpool.tile([P, R], BF, name=f"qt{m}")
                          for m in range(KT)]
                    for m in range(KT):
                        pq = [psum.tile([P, CH], F32, tag="mm", bufs=3,
                                        name="psq") for _ in range(NIC)]
                        for d_ in range(KT):
                            for ic in range(NIC):
                                nc.tensor.matmul(
                                    pq[ic], WqT[d_][:, m * P:(m + 1) * P],
                                    HcT[d_][:, ic * CH:(ic + 1) * CH],
                                    start=(d_ == 0), stop=(d_ == KT - 1))
                        for ic in range(NIC):
                            nc.scalar.activation(
                                QT[m][:, ic * CH:(ic + 1) * CH], pq[ic],
                                AF.Identity, bias=bq_sb[m])
                    for k in range(KT):
                        pq2 = [psum.tile([P, CH], F32, tag="mm", bufs=3,
                                         name="psq2") for _ in range(NIC)]
                        for m in range(KT):
                            for ic in range(NIC):
                                nc.tensor.matmul(
                                    pq2[ic], Wk[m][:, k * P:(k + 1) * P],
                                    QT[m][:, ic * CH:(ic + 1) * CH],
                                    start=(m == 0), stop=(m == KT - 1))
                        for ic in range(NIC):
                            nc.scalar.activation(
                                q2f8[:, k:k + 1, ic * CH:(ic + 1) * CH],
                                pq2[ic], AF.Copy, scale=Q2SCALE)

                # -- distance groups, software pipelined ---------------------
                for jg in range(1, NG):
                    if jg + 1 < NG:
                        tts_next = load_group(jg + 1)
                        augg_next = xx_chain(jg + 1, tts_next)
                    else:
                        tts_next = augg_next = None
                    emit_ht8(2)
                    if jg == 1:
                        emit_tailw()
                    d2_group(jg, tts_cur, augg_cur)
                    tts_cur, augg_cur = tts_next, augg_next

            with tc.tile_pool(name="scl", bufs=1, space="DRAM") as dram:
                scl_dram = dram.tile([R, 1], F32, name="scldram")
                for it in range(NIT):
                    red = early.tile([P, 1], F32, name=f"red{it}")
                    nc.vector.reduce_sum(red, dsum[it],
                                         axis=mybir.AxisListType.X)
                    mcol = early.tile([P, 1], F32, name=f"mcol{it}")
                    nc.vector.tensor_scalar(mcol, red, 1.0 / N, None,
                                            op0=ALU.mult)
                    ucol = early.tile([P, 1], F32, name=f"ucol{it}")
                    nc.vector.reciprocal(ucol, mcol)
                    # measured mean of sqrt(dist2+16) = true mean + 8/m +
                    # diag excess 4/N; scale = 1/(1 + m - 8/m - 0.000488)
                    uc2 = early.tile([P, 1], F32, name=f"uc2{it}")
                    nc.vector.tensor_scalar(uc2, ucol, -8.0, 0.999512,
                                            op0=ALU.mult, op1=ALU.add)
                    tmp = early.tile([P, 1], F32, name=f"sctmp{it}")
                    nc.vector.tensor_add(tmp, mcol, uc2)
                    scol = early.tile([P, 1], F32, name=f"scol{it}")
                    nc.vector.reciprocal(scol, tmp)
                    nc.sync.dma_start(scl_dram[it * P:(it + 1) * P, :], scol)
                nc.sync.dma_start(
                    scl_row, scl_dram.rearrange("(a p) c -> a (p c)", a=1))
                nc.vector.tensor_copy(scl_b, scl_row)


        # ---- attention passes: pipelined logits(pair s) | G/rowsum(s-2) ----
        def attention_pass(ic):
            csl = slice(ic * CH, (ic + 1) * CH)
            g_ps = [psum.tile([P, CH], F32, tag=f"g{d_}", name=f"gps{d_}")
                    for d_ in range(KT)]
            rs_ps = psum.tile([P, CH], F32, tag="rowps", name="rsps")
            # two-deep pair pipeline: G/rowsum lag the logits by 2 pairs so
            # the exp of pair s-2 is long done when its G matmuls issue
            pipe = []  # [(e2_t, h2_t, s), ...]

            def g_mm(lag, k, stop=False):
                nc.tensor.matmul(g_ps[k], lag[1][:, :, k * P:(k + 1) * P],
                                 lag[0], start=(lag[2] == 0), stop=stop,
                                 perf_mode=DR)

            for s in range(NPAIR):
                h2_t = h_pool.tile([P, 2, D], F8, tag="h", name="h2t")
                nc.sync.dma_start(h2_t[:, 0:1, :],
                                  io["Hf8"][(2 * s) * P:(2 * s + 1) * P, :])
                nc.sync.dma_start(h2_t[:, 1:2, :],
                                  io["Hf8"][(2 * s + 1) * P:(2 * s + 2) * P, :])
                e2_t = e_pool.tile([P, 2, CH], F8, tag="e", name="e2t")
                st_a = psum.tile([P, CH], F32, tag="mm", bufs=3, name="sta")
                st_b = psum.tile([P, CH], F32, tag="mm", bufs=3, name="stb")
                lag = pipe[0] if len(pipe) == 2 else None
                nc.tensor.matmul(st_a, ht8[:, 0:2, (2 * s) * P:(2 * s + 1) * P],
                                 q2f8[:, 0:2, csl], start=True, stop=False,
                                 perf_mode=DR)
                if lag is not None:
                    g_mm(lag, 0)
                nc.tensor.matmul(st_a, ht8[:, 2:4, (2 * s) * P:(2 * s + 1) * P],
                                 q2f8[:, 2:4, csl], start=False, stop=True,
                                 perf_mode=DR)
                if lag is not None:
                    g_mm(lag, 1)
                nc.scalar.activation(e2_t[:, 0:1, :], st_a, AF.Exp,
                                     scale=EXPSCALE, bias=expb_col)
                nc.tensor.matmul(st_b,
                                 ht8[:, 0:2, (2 * s + 1) * P:(2 * s + 2) * P],
                                 q2f8[:, 0:2, csl], start=True, stop=False,
                                 perf_mode=DR)
                if lag is not None:
                    g_mm(lag, 2)
                nc.tensor.matmul(st_b,
                                 ht8[:, 2:4, (2 * s + 1) * P:(2 * s + 2) * P],
                                 q2f8[:, 2:4, csl], start=False, stop=True,
                                 perf_mode=DR)
                if lag is not None:
                    g_mm(lag, 3)
                    nc.tensor.matmul(rs_ps, onesw2, lag[0],
                                     start=(lag[2] == 0), stop=False,
                                     perf_mode=DR)
                    pipe.pop(0)
                nc.scalar.activation(e2_t[:, 1:2, :], st_b, AF.Exp,
                                     scale=EXPSCALE, bias=expb_col)
                pipe.append((e2_t, h2_t, s))
            for (e2_t, h2_t, s) in pipe:
                last = s == NPAIR - 1
                for k in range(KT):
                    nc.tensor.matmul(g_ps[k], h2_t[:, :, k * P:(k + 1) * P],
                                     e2_t, start=(s == 0), stop=last,
                                     perf_mode=DR)
                nc.tensor.matmul(rs_ps, onesw2, e2_t, start=(s == 0),
                                 stop=last, perf_mode=DR)
            # drain accumulators promptly so the next pass can claim the banks
            for d_ in range(KT):
                nc.scalar.activation(GT[d_][:, csl], g_ps[d_], AF.Copy)
            nc.vector.tensor_copy(rs_row[0:1, csl], rs_ps[0:1, :])

        def tail(ic):
            csl = slice(ic * CH, (ic + 1) * CH)
            nc.vector.reciprocal(sn_row[0:1, csl], rs_row[0:1, csl])
            nc.vector.tensor_mul(sn_row[0:1, csl], sn_row[0:1, csl],
                                 scl_row[0:1, csl])
            ps_snb = psum.tile([P, CH], F32, tag="mm", bufs=3, name="pssnb")
            nc.tensor.matmul(ps_snb, ones_f1, sn_row[0:1, csl],
                             start=True, stop=True)
            nc.vector.tensor_copy(SNB[:, csl], ps_snb)
            for d_ in range(KT):
                nc.vector.tensor_mul(GT[d_][:, csl], GT[d_][:, csl],
                                     SNB[:, csl])
            # Y^T = Wv Gn^T + (bv x scale): two m-chains in flight
            for m0 in range(0, KT, 2):
                py = [psum.tile([P, CH], F32, tag="mm", bufs=3, name="psy")
                      for _ in range(2)]
                for d_ in range(KT):
                    for u in range(2):
                        m = m0 + u
                        nc.tensor.matmul(py[u], WvT[d_][:, m * P:(m + 1) * P],
                                         GT[d_][:, csl],
                                         start=(d_ == 0), stop=False)
                for u in range(2):
                    m = m0 + u
                    nc.tensor.matmul(py[u], bv_row[0:1, m * P:(m + 1) * P],
                                     scl_b[0:1, csl], start=False, stop=True)
                for u in range(2):
                    m = m0 + u
                    nc.scalar.activation(YT[m][:, csl], py[u], AF.Copy)
            # out = Y Wo^T + bo for this chunk's 4 i-tiles, chains in pairs
            for it0 in range(ic * 4, (ic + 1) * 4, 2):
                po = [psum.tile([P, CH], F32, tag="mm", bufs=3, name="pso")
                      for _ in range(2)]
                for m in range(KT):
                    for u in range(2):
                        it = it0 + u
                        nc.tensor.matmul(po[u], YT[m][:, it * P:(it + 1) * P],
                                         WoT[m], start=(m == 0), stop=False)
                for u in range(2):
                    nc.tensor.matmul(po[u], ones_b1, bo_row,
                                     start=False, stop=True)
                for u in range(2):
                    it = it0 + u
                    o_t = o_pool.tile([P, D], F32, tag="o", name="ot")
                    nc.scalar.activation(o_t, po[u], AF.Copy)
                    nc.sync.dma_start(io["OUT"][it * P:(it + 1) * P, :], o_t)

        attention_pass(0)
        attention_pass(1)
        tail(0)
        tail(1)


_NC_CACHE = None


def _build():
    global _NC_CACHE
    if _NC_CACHE is not None:
        return _NC_CACHE
    nc = bacc.Bacc("TRN2", target_bir_lowering=False, debug=False,
                   enable_asserts=False, num_devices=NCORES)
    io = {
        "HTf8": nc.dram_tensor("HTf8", [D, N], F8, kind="ExternalInput").ap(),
        "Hf8": nc.dram_tensor("Hf8", [N, D], F8, kind="ExternalInput").ap(),
        "TTf8": nc.dram_tensor("TTf8", [D, N], F8, kind="ExternalInput").ap(),
        "TcTf8": nc.dram_tensor("TcTf8", [D, R], F8,
                                kind="ExternalInput").ap(),
        "HcTb": nc.dram_tensor("HcTb", [D, R], BF, kind="ExternalInput").ap(),
        "WqTb": nc.dram_tensor("WqTb", [D, D], BF, kind="ExternalInput").ap(),
        "Wkb": nc.dram_tensor("Wkb", [D, D], BF, kind="ExternalInput").ap(),
        "WvTb": nc.dram_tensor("WvTb", [D, D], BF, kind="ExternalInput").ap(),
        "WoTb": nc.dram_tensor("WoTb", [D, D], BF, kind="ExternalInput").ap(),
        "bqf": nc.dram_tensor("bqf", [D, 1], F32, kind="ExternalInput").ap(),
        "bvb": nc.dram_tensor("bvb", [1, D], BF, kind="ExternalInput").ap(),
        "bob": nc.dram_tensor("bob", [1, D], BF, kind="ExternalInput").ap(),
        "OUT": nc.dram_tensor("OUT", [R, D], F32, kind="ExternalOutput").ap(),
    }
    with tile.TileContext(nc) as tc:
        _emit(tc, io)
    nc.compile()
    _NC_CACHE = nc
    return nc


LAST_RESULTS = None


def _to_f8(a):
    return np.clip(a, -240.0, 240.0).astype(f8e4)


def kernel(H, T, Wq, bq, Wk, bk, Wv, bv, Wo, bo):
    global LAST_RESULTS
    H = np.ascontiguousarray(np.asarray(H, np.float32))
    T = np.ascontiguousarray(np.asarray(T, np.float32))

    HT = np.ascontiguousarray(H.T)
    TT = np.ascontiguousarray(T.T)
    HTb = HT.astype(bf16)
    shared = {
        "HTf8": _to_f8(HT),
        "Hf8": _to_f8(H),
        "TTf8": _to_f8(TT),
        "WqTb": np.ascontiguousarray(np.asarray(Wq, np.float32).T).astype(bf16),
        "Wkb": np.ascontiguousarray(np.asarray(Wk, np.float32)).astype(bf16),
        "WvTb": np.ascontiguousarray(np.asarray(Wv, np.float32).T).astype(bf16),
        "WoTb": np.ascontiguousarray(np.asarray(Wo, np.float32).T).astype(bf16),
        "bqf": np.asarray(bq, np.float32).reshape(D, 1).copy(),
        "bvb": np.asarray(bv, np.float32).reshape(1, D).astype(bf16),
        "bob": np.asarray(bo, np.float32).reshape(1, D).astype(bf16),
    }
    in_maps = []
    for c in range(NCORES):
        m = dict(shared)
        m["TcTf8"] = np.ascontiguousarray(shared["TTf8"][:, c * R:(c + 1) * R])
        m["HcTb"] = np.ascontiguousarray(HTb[:, c * R:(c + 1) * R])
        in_maps.append(m)

    nc = _build()
    res = bass_utils.run_bass_kernel_spmd(nc, in_maps,
                                          core_ids=list(range(NCORES)))
    LAST_RESULTS = res
    out = np.concatenate([res.results[c]["OUT"] for c in range(NCORES)],
                         axis=0)
    return np.ascontiguousarray(out.astype(np.float32))
